# revision 26
# baseline (speedup 1.0000x reference)
"""Trainium2 Bass kernel for nn_AdaptiveInterventionEngine.

Data-parallel over batch across 8 NeuronCores. Feature-major (transposed)
activation layout on device so every weight matrix is used as lhsT exactly
as stored. Encoder matmuls bf16 (fp32 PSUM accumulate), LayerNorm stats in
fp32 via ones-vector matmuls on the PE, recurrence in fp32 via float32r
matmuls. The per-step halt decision is decoupled: the device computes the
full 6-step trajectory plus per-element halter logits; the host reduces the
logit means, derives steps_used and selects the matching z state.
"""

import math

import numpy as np
import ml_dtypes

import concourse.bass as bass
import concourse.mybir as mybir
import concourse.tile as tile
from concourse import bacc
from concourse.bass import ts
from concourse.bass_utils import run_bass_kernel_spmd

BF16NP = ml_dtypes.bfloat16
F32 = mybir.dt.float32
F32R = mybir.dt.float32r
BF16 = mybir.dt.bfloat16
AF = mybir.ActivationFunctionType
OP = mybir.AluOpType

NCORES = 8
B = 32768
BSH = B // NCORES          # 4096 batch rows per core
DC = 512                   # causal dim
DA = 1024                  # action dim
HID = 2048                 # hidden dim
ENC_D = DC // 4            # 128
STEPS = 6
NT = 256                   # batch columns per tile
NTILES = BSH // NT         # 16
KA = DA // 128             # 8  action k-chunks
KH = HID // 128            # 16 hidden k-chunks
KC = DC // 128             # 4  causal k-chunks

TRACE = False
LAST_EXEC_NS = None
LAST_RESULTS = None


def _sin_enc(step_f, d):
    div = np.exp(np.arange(0, d, 2, dtype=np.float64) * (-(math.log(10000.0) / d)))
    pe = np.zeros((d,), dtype=np.float64)
    pe[0::2] = np.sin(step_f * div[: (d + 1) // 2])
    pe[1::2] = np.cos(step_f * div[: d // 2])
    return pe


def _pcol(v):
    """[n*128] vector -> [128, n] (partition, m-chunk) layout, f32."""
    v = np.asarray(v, np.float32)
    n = v.shape[0] // 128
    return np.ascontiguousarray(v.reshape(n, 128).T)


def build_nc():
    nc = bacc.Bacc(
        "TRN2", target_bir_lowering=False, debug=False, num_devices=NCORES
    )

    def din(name, shape, dt):
        return nc.dram_tensor(name, shape, dt, kind="ExternalInput").ap()

    def dout(name, shape, dt):
        return nc.dram_tensor(name, shape, dt, kind="ExternalOutput").ap()

    actT_d = din("actionT", [DA, BSH], BF16)
    zT_d = din("zT", [DC, BSH], F32)
    zbf_d = din("zTbf", [DC, BSH], BF16)
    w1_d = din("w1", [DA, HID], BF16)
    w2_d = din("w2", [HID, HID], BF16)
    wmv_d = din("wmv", [HID, 2 * DC], BF16)
    adj_d = din("adj", [DC, DC], BF16)
    wh1_d = din("wh1a", [DC, DC], BF16)
    wh2_d = din("wh2", [DC, 1], BF16)
    b1g_d = din("b1g", [128, 3 * KH], F32)    # b1 | g1 | be1
    b2g_d = din("b2g", [128, 3 * KH], F32)    # b2 | g2 | be2
    bmv_d = din("bmv", [128, 2 * KC], F32)    # -bm | bv
    cst_d = din("cst", [128, STEPS * KC], F32)  # halter per-step bias

    maskT_o = dout("maskT", [DC, BSH], F32)
    epsT_o = dout("epsT", [DC, BSH], F32)
    zallT_o = dout("zallT", [STEPS, DC, BSH], BF16)
    y_o = dout("yout", [STEPS, BSH], F32)

    with tile.TileContext(nc) as tc:
        build_kernel(
            nc, tc,
            actT_d, zT_d, zbf_d, w1_d, w2_d, wmv_d, adj_d, wh1_d, wh2_d,
            b1g_d, b2g_d, bmv_d, cst_d,
            maskT_o, epsT_o, zallT_o, y_o,
        )
    nc.compile()
    return nc


def build_kernel(nc, tc, actT_d, zT_d, zbf_d, w1_d, w2_d, wmv_d, adj_d, wh1_d, wh2_d,
                 b1g_d, b2g_d, bmv_d, cst_d, maskT_o, epsT_o, zallT_o, y_o):
    from contextlib import ExitStack

    ctx = ExitStack()
    with ctx:
        singles = ctx.enter_context(tc.tile_pool(name="singles", bufs=1))
        w2p = ctx.enter_context(tc.tile_pool(name="w2p", bufs=2))
        actp = ctx.enter_context(tc.tile_pool(name="actp", bufs=2))
        tp = ctx.enter_context(tc.tile_pool(name="tp", bufs=2))
        h1p = ctx.enter_context(tc.tile_pool(name="h1p", bufs=2))
        h2p = ctx.enter_context(tc.tile_pool(name="h2p", bufs=2))
        sqp = ctx.enter_context(tc.tile_pool(name="sqp", bufs=1))
        stp = ctx.enter_context(tc.tile_pool(name="stp", bufs=5))
        bcp = ctx.enter_context(tc.tile_pool(name="bcp", bufs=2))
        nscp = ctx.enter_context(tc.tile_pool(name="nscp", bufs=2))
        zinp = ctx.enter_context(tc.tile_pool(name="zinp", bufs=2))
        zbfp = ctx.enter_context(tc.tile_pool(name="zbfp", bufs=1))
        m1p = ctx.enter_context(tc.tile_pool(name="m1p", bufs=2))
        valp = ctx.enter_context(tc.tile_pool(name="valp", bufs=1))
        epsp = ctx.enter_context(tc.tile_pool(name="epsp", bufs=1))
        maskp = ctx.enter_context(tc.tile_pool(name="maskp", bufs=1))
        fscp = ctx.enter_context(tc.tile_pool(name="fscp", bufs=4))
        cp = ctx.enter_context(tc.tile_pool(name="cp", bufs=2))
        zp = ctx.enter_context(tc.tile_pool(name="zp", bufs=4))
        dltp = ctx.enter_context(tc.tile_pool(name="dltp", bufs=1))
        gp = ctx.enter_context(tc.tile_pool(name="gp", bufs=1))
        yp = ctx.enter_context(tc.tile_pool(name="yp", bufs=1))
        mmps = ctx.enter_context(tc.tile_pool(name="mmps", bufs=5, space="PSUM"))
        auxps = ctx.enter_context(tc.tile_pool(name="auxps", bufs=3, space="PSUM"))

        # ---- resident weights -------------------------------------------
        w1_s = singles.tile([128, KA, HID], BF16, name="w1_s")
        nc.sync.dma_start(out=w1_s, in_=w1_d.rearrange("(k p) m -> p k m", p=128))
        wmv_s = singles.tile([128, KH, 2 * DC], BF16, name="wmv_s")
        nc.sync.dma_start(out=wmv_s, in_=wmv_d.rearrange("(k p) m -> p k m", p=128))
        adj_s = singles.tile([128, KC, DC], BF16, name="adj_s")
        nc.sync.dma_start(out=adj_s, in_=adj_d.rearrange("(k p) m -> p k m", p=128))
        wh1_s = singles.tile([128, KC, DC], BF16, name="wh1_s")
        nc.sync.dma_start(out=wh1_s, in_=wh1_d.rearrange("(k p) m -> p k m", p=128))
        wh2_s = singles.tile([128, KC], BF16, name="wh2_s")
        nc.sync.dma_start(out=wh2_s, in_=wh2_d.rearrange("(k p) o -> p (k o)", p=128))
        b1g_s = singles.tile([128, 3 * KH], F32, name="b1g_s")
        nc.sync.dma_start(out=b1g_s, in_=b1g_d)
        b2g_s = singles.tile([128, 3 * KH], F32, name="b2g_s")
        nc.sync.dma_start(out=b2g_s, in_=b2g_d)
        bmv_s = singles.tile([128, 2 * KC], F32, name="bmv_s")
        nc.sync.dma_start(out=bmv_s, in_=bmv_d)
        cst_s = singles.tile([128, STEPS * KC], F32, name="cst_s")
        nc.sync.dma_start(out=cst_s, in_=cst_d)
        ones_bf = singles.tile([128, 1], BF16, name="ones_bf")
        nc.vector.memset(ones_bf, 1.0)
        ones_r = singles.tile([1, 128], BF16, name="ones_r")
        nc.vector.memset(ones_r, 1.0)
        epsb = singles.tile([1, 1], F32, name="epsb")
        nc.vector.memset(epsb, 1e-5)

        actT_r = actT_d.rearrange("(k p) n -> p k n", p=128)
        zT_r = zT_d.rearrange("(k p) n -> p k n", p=128)
        zbf_r = zbf_d.rearrange("(k p) n -> p k n", p=128)
        maskT_r = maskT_o.rearrange("(m p) n -> p m n", p=128)
        epsT_r = epsT_o.rearrange("(m p) n -> p m n", p=128)
        zallT_r = zallT_o.rearrange("t (m p) n -> t p m n", p=128)

        def bcast2(ap2d, n):
            return bass.AP(tensor=ap2d.tensor, offset=ap2d.offset,
                           ap=[ap2d.ap[0], [0, n], ap2d.ap[1]])

        def mm_quarters(M, K, lhsT_fn, rhs_fn, drain_fn, q=4):
            """out_mtile[m] = sum_k lhsT(k,m).T @ rhs(k); M,K in 128-tiles."""
            for q0 in range(0, M, q):
                nq = min(q, M - q0)
                pss = []
                for mi in range(nq):
                    ps = mmps.tile([128, NT], F32, tag="mm", name="ps")
                    pss.append(ps)
                for k in range(K):
                    for mi in range(nq):
                        nc.tensor.matmul(
                            pss[mi], lhsT_fn(k, q0 + mi), rhs_fn(k),
                            start=(k == 0), stop=(k == K - 1),
                        )
                for mi in range(nq):
                    drain_fn(q0 + mi, pss[mi])

        def layer_stats(t_s, ktiles):
            """sum/sumsq over partition-axis features into one packed psum."""
            st_ps = auxps.tile([1, 2, NT], F32, tag="aux", name="st_ps")
            for kp in range(0, ktiles, 2):
                slw = t_s[:, ts(kp // 2, 2 * NT)]
                sq_t = sqp.tile([128, 2 * NT], BF16, tag="sq", name="sq_t")
                nc.gpsimd.tensor_tensor(sq_t, slw, slw, op=OP.mult)
                for dk in range(2):
                    k = kp + dk
                    nc.tensor.matmul(st_ps[:, 0, :], ones_bf, t_s[:, ts(k, NT)],
                                     start=(k == 0), stop=(k == ktiles - 1))
                    nc.tensor.matmul(st_ps[:, 1, :], ones_bf, sq_t[:, ts(dk, NT)],
                                     start=(k == 0), stop=(k == ktiles - 1))
            return st_ps

        def layer_norm_apply(st_ps, t_s, h_s, ktiles, gvec_off, bias_off, bg_s):
            inv_n = 1.0 / (ktiles * 128)
            mean_s = stp.tile([1, NT], F32, tag="st", name="mean_s")
            nc.scalar.activation(mean_s, st_ps[:, 0, :], AF.Copy, scale=inv_n)
            ex2_s = stp.tile([1, NT], F32, tag="st", name="ex2_s")
            nc.scalar.activation(ex2_s, st_ps[:, 1, :], AF.Identity,
                                 bias=epsb, scale=inv_n)
            msq_s = stp.tile([1, NT], F32, tag="st", name="msq_s")
            nc.vector.tensor_tensor(msq_s, mean_s, mean_s, op=OP.mult)
            vpe_s = stp.tile([1, NT], F32, tag="st", name="vpe_s")
            nc.vector.tensor_tensor(vpe_s, ex2_s, msq_s, op=OP.subtract)
            sd_s = stp.tile([1, NT], F32, tag="st", name="sd_s")
            nc.scalar.activation(sd_s, vpe_s, AF.Sqrt)
            rstd_s = stp.tile([1, NT], BF16, tag="st", name="rstd_s")
            nmr_s = stp.tile([1, NT], BF16, tag="st", name="nmr_s")
            with nc.allow_low_precision(reason="LN scale vectors feed bf16 bcast matmul"):
                nc.vector.reciprocal(rstd_s, sd_s)
                nc.vector.scalar_tensor_tensor(
                    nmr_s, mean_s, -1.0, rstd_s, op0=OP.mult, op1=OP.mult)

            bc_ps = mmps.tile([128, NT], F32, tag="mm", name="bc_ps")
            nc.tensor.matmul(bc_ps, ones_r, rstd_s, start=True, stop=True)
            rstd_b = bcp.tile([128, NT], F32, tag="bc", name="rstd_b")
            nc.scalar.activation(rstd_b, bc_ps, AF.Copy)
            bc2_ps = mmps.tile([128, NT], F32, tag="mm", name="bc2_ps")
            nc.tensor.matmul(bc2_ps, ones_r, nmr_s, start=True, stop=True)
            nmr_b = bcp.tile([128, NT], F32, tag="bc", name="nmr_b")
            nc.scalar.activation(nmr_b, bc2_ps, AF.Copy)

            for kp in range(0, ktiles, 2):
                t3 = t_s[:, ts(kp // 2, 2 * NT)].rearrange("p (b n) -> p b n", b=2)
                x1_t = nscp.tile([128, 2, NT], F32, tag="nsc", name="x1_t")
                nc.vector.tensor_tensor(x1_t, t3, bcast2(rstd_b, 2), op=OP.mult)
                nc.vector.tensor_tensor(x1_t, x1_t, bcast2(nmr_b, 2), op=OP.add)
                for dk in range(2):
                    k = kp + dk
                    nc.scalar.activation(
                        h_s[:, ts(k, NT)], x1_t[:, dk, :], AF.Gelu,
                        bias=bg_s[:, bias_off + k:bias_off + k + 1],
                        scale=bg_s[:, gvec_off + k:gvec_off + k + 1],
                    )

        # ---- per batch-tile pipeline ------------------------------------
        w2_r = w2_d.rearrange("(k p) m -> p k m", p=128)

        def phase1(j, ld):
            """L1, stats1 (inputs preloaded)."""
            cols = ld["cols"]
            a_s = ld["a_s"]
            zin_s = ld["zin_s"]
            t1_s = tp.tile([128, KH * NT], BF16, tag="t", name="t1_s")

            def l1_drain(m, ps):
                nc.scalar.activation(t1_s[:, ts(m, NT)], ps, AF.Identity,
                                     bias=b1g_s[:, m:m + 1])

            mm_quarters(
                KH, KA,
                lambda k, m: w1_s[:, k, ts(m, 128)],
                lambda k: a_s[:, ts(k, NT)],
                l1_drain,
            )
            st1 = layer_stats(t1_s, KH)
            return dict(cols=cols, zin_s=zin_s, t1_s=t1_s, st1=st1)

        def phase2(st):
            """LN1 chain+norm, L2, stats2, LN2 chain+norm."""
            h1_s = h1p.tile([128, KH * NT], BF16, tag="h1", name="h1_s")
            layer_norm_apply(st["st1"], st["t1_s"], h1_s, KH, KH, 2 * KH, b1g_s)

            t2_s = tp.tile([128, KH * NT], BF16, tag="t", name="t2_s")
            for q0 in range(0, KH, 4):
                pss = []
                for mi in range(4):
                    ps = mmps.tile([128, NT], F32, tag="mm", name="ps")
                    pss.append(ps)
                for kg in range(0, KH, 4):
                    wt = w2p.tile([128, 4, 4 * 128], BF16, tag="w2", name="wt")
                    nc.sync.dma_start(
                        out=wt,
                        in_=w2_r[:, kg:kg + 4, q0 * 128:(q0 + 4) * 128],
                    )
                    for dk in range(4):
                        k = kg + dk
                        for mi in range(4):
                            nc.tensor.matmul(
                                pss[mi], wt[:, dk, ts(mi, 128)],
                                h1_s[:, ts(k, NT)],
                                start=(k == 0), stop=(k == KH - 1),
                            )
                for mi in range(4):
                    m = q0 + mi
                    nc.scalar.activation(t2_s[:, ts(m, NT)], pss[mi], AF.Identity,
                                         bias=b2g_s[:, m:m + 1])
            st2 = layer_stats(t2_s, KH)
            h2_s = h2p.tile([128, KH * NT], BF16, tag="h2", name="h2_s")
            layer_norm_apply(st2, t2_s, h2_s, KH, KH, 2 * KH, b2g_s)
            st["h2_s"] = h2_s

        def phase3(st):
            """wmv, mask out, eps, z0/c."""
            cols = st["cols"]
            zin_s = st["zin_s"]
            h2_s = st["h2_s"]
            zbf_s = zbfp.tile([128, KC * NT], BF16, tag="zbf", name="zbf_s")
            nc.sync.dma_start(
                out=zbf_s.rearrange("p (k n) -> p k n", k=KC),
                in_=zbf_r[:, :, cols],
            )
            m1_s = m1p.tile([128, KC * NT], F32, tag="m1", name="m1_s")
            val_s = valp.tile([128, KC * NT], F32, tag="val", name="val_s")
            mask_s = maskp.tile([128, KC * NT], F32, tag="mask", name="mask_s")

            def mv_drain(m, ps):
                if m < KC:
                    nc.scalar.activation(m1_s[:, ts(m, NT)], ps, AF.Sigmoid,
                                         bias=bmv_s[:, m:m + 1], scale=-1.0)
                else:
                    mm = m - KC
                    nc.scalar.activation(val_s[:, ts(mm, NT)], ps, AF.Identity,
                                         bias=bmv_s[:, KC + mm:KC + mm + 1])

            mm_quarters(
                2 * KC, KH,
                lambda k, m: wmv_s[:, k, ts(m, 128)],
                lambda k: h2_s[:, ts(k, NT)],
                mv_drain,
            )
            nc.vector.tensor_scalar(mask_s, m1_s, -1.0, 1.0,
                                    op0=OP.mult, op1=OP.add)
            nc.sync.dma_start(
                out=maskT_r[:, :, cols],
                in_=mask_s.rearrange("p (m n) -> p m n", m=KC),
            )

            eps_s = epsp.tile([128, KC * NT], F32, tag="eps", name="eps_s")

            def eps_drain(m, ps):
                nc.vector.tensor_tensor(
                    eps_s[:, ts(m, NT)], zin_s[:, ts(m, NT)], ps, op=OP.subtract)

            mm_quarters(
                KC, KC,
                lambda k, m: adj_s[:, k, ts(m, 128)],
                lambda k: zbf_s[:, ts(k, NT)],
                eps_drain,
            )
            nc.sync.dma_start(
                out=epsT_r[:, :, cols],
                in_=eps_s.rearrange("p (m n) -> p m n", m=KC),
            )

            z0_s = zp.tile([128, KC * NT], BF16, tag="z", name="z0_s")
            c_s = cp.tile([128, KC * NT], F32, tag="c", name="c_s")
            for m in range(KC):
                sl = ts(m, NT)
                d1 = fscp.tile([128, NT], F32, tag="fsc2", name="d1")
                nc.gpsimd.tensor_tensor(d1, zin_s[:, sl], val_s[:, sl],
                                        op=OP.subtract)
                p1 = fscp.tile([128, NT], F32, tag="fsc2", name="p1")
                nc.vector.tensor_tensor(p1, d1, m1_s[:, sl], op=OP.mult)
                z0f = fscp.tile([128, NT], F32, tag="fsc2", name="z0f")
                nc.vector.tensor_tensor(z0f, val_s[:, sl], p1, op=OP.add)
                nc.gpsimd.tensor_copy(z0_s[:, sl], z0f)
                e2 = fscp.tile([128, NT], F32, tag="fsc2", name="e2")
                nc.gpsimd.tensor_tensor(e2, eps_s[:, sl], z0f, op=OP.subtract)
                p2 = fscp.tile([128, NT], F32, tag="fsc2", name="p2")
                nc.vector.tensor_tensor(p2, e2, m1_s[:, sl], op=OP.mult)
                nc.vector.tensor_tensor(c_s[:, sl], z0f, p2, op=OP.add)
            st["m1_s"] = m1_s
            st["c_s"] = c_s
            st["z_cur"] = z0_s

        def tile_step(st, t):
            cols = st["cols"]
            m1_s = st["m1_s"]
            c_s = st["c_s"]
            z_cur = st["z_cur"]
            z_new = zp.tile([128, KC * NT], BF16, tag="z", name="z_new")
            delta_s = dltp.tile([128, KC * NT], BF16, tag="dlt", name="delta_s")

            def z_drain(m, ps):
                sl = ts(m, NT)
                p1 = fscp.tile([128, NT], F32, tag="fsc2", name="zp1")
                nc.vector.tensor_tensor(p1, ps, m1_s[:, sl], op=OP.mult)
                nc.vector.tensor_tensor(z_new[:, sl], c_s[:, sl], p1, op=OP.add)
                ds_ = fscp.tile([128, NT], BF16, tag="dsc", name="ds_")
                nc.vector.tensor_tensor(ds_, z_new[:, sl], z_cur[:, sl],
                                        op=OP.subtract)
                nc.scalar.activation(delta_s[:, sl], ds_, AF.Abs)

            mm_quarters(
                KC, KC,
                lambda k, m: adj_s[:, k, ts(m, 128)],
                lambda k: z_cur[:, ts(k, NT)],
                z_drain,
            )
            nc.sync.dma_start(
                out=zallT_r[t, :, :, cols],
                in_=z_new.rearrange("p (m n) -> p m n", m=KC),
            )

            g_s = gp.tile([128, KC * NT], BF16, tag="g", name="g_s")

            def g_drain(m, ps):
                nc.scalar.activation(g_s[:, ts(m, NT)], ps, AF.Gelu,
                                     bias=cst_s[:, t * KC + m:t * KC + m + 1])

            mm_quarters(
                KC, KC,
                lambda k, m: wh1_s[:, k, ts(m, 128)],
                lambda k: delta_s[:, ts(k, NT)],
                g_drain,
            )

            y_ps = auxps.tile([1, NT], F32, tag="aux", name="y_ps")
            for k in range(KC):
                nc.tensor.matmul(
                    y_ps, wh2_s[:, k:k + 1], g_s[:, ts(k, NT)],
                    start=(k == 0), stop=(k == KC - 1),
                )
            y_row = yp.tile([1, NT], F32, tag="y", name="y_row")
            nc.scalar.activation(y_row, y_ps, AF.Copy)
            nc.sync.dma_start(out=y_o[t:t + 1, cols], in_=y_row)
            st["z_cur"] = z_new

        def preload(j):
            cols = slice(j * NT, (j + 1) * NT)
            a_s = actp.tile([128, KA * NT], BF16, tag="act", name="a_s")
            nc.sync.dma_start(
                out=a_s.rearrange("p (k n) -> p k n", k=KA),
                in_=actT_r[:, :, cols],
            )
            zin_s = zinp.tile([128, KC * NT], F32, tag="zin", name="zin_s")
            nc.sync.dma_start(
                out=zin_s.rearrange("p (k n) -> p k n", k=KC),
                in_=zT_r[:, :, cols],
            )
            return dict(cols=cols, a_s=a_s, zin_s=zin_s)

        ld = preload(0)
        for j in range(NTILES):
            sj = phase1(j, ld)
            phase2(sj)
            phase3(sj)
            if j + 1 < NTILES:
                ld = preload(j + 1)
            for t in range(STEPS):
                tile_step(sj, t)


_built = None


def _get_nc():
    global _built
    if _built is None:
        _built = build_nc()
    return _built


def kernel(**inputs):
    global LAST_EXEC_NS, LAST_RESULTS
    inp = {k: np.asarray(v) for k, v in inputs.items()}

    w1 = inp["w1"].astype(BF16NP)
    w2 = inp["w2"].astype(BF16NP)
    wmv = np.concatenate([inp["wm"], inp["wv"]], axis=1).astype(BF16NP)
    adj = inp["adjacency"].astype(BF16NP)
    wh1a = inp["wh1"][:DC].astype(BF16NP)
    wh2 = inp["wh2"].astype(BF16NP)
    b1g = np.concatenate(
        [_pcol(inp["b1"]), _pcol(inp["g1"]), _pcol(inp["be1"])], axis=1)
    b2g = np.concatenate(
        [_pcol(inp["b2"]), _pcol(inp["g2"]), _pcol(inp["be2"])], axis=1)
    bmv = np.concatenate([_pcol(-inp["bm"]), _pcol(inp["bv"])], axis=1)
    cst_cols = []
    wh1b = inp["wh1"][DC:].astype(np.float64)
    bh1 = inp["bh1"].astype(np.float64)
    for t in range(STEPS):
        enc = _sin_enc(float(t), ENC_D)
        cst_cols.append(_pcol((enc @ wh1b + bh1).astype(np.float32)))
    cst = np.concatenate(cst_cols, axis=1)

    shared = dict(w1=w1, w2=w2, wmv=wmv, adj=adj, wh1a=wh1a, wh2=wh2,
                  b1g=b1g, b2g=b2g, bmv=bmv, cst=cst)

    in_maps = []
    for c in range(NCORES):
        rows = slice(c * BSH, (c + 1) * BSH)
        m = dict(shared)
        m["actionT"] = np.ascontiguousarray(inp["action"][rows].T).astype(BF16NP)
        zt = np.ascontiguousarray(inp["z"][rows].T.astype(np.float32))
        m["zT"] = zt
        m["zTbf"] = zt.astype(BF16NP)
        in_maps.append(m)

    nc = _get_nc()
    res = run_bass_kernel_spmd(
        nc, in_maps, core_ids=list(range(NCORES)), trace=TRACE
    )
    LAST_EXEC_NS = res.exec_time_ns
    LAST_RESULTS = res
    outs = res.results

    # ---- host-side unshard + halt decision --------------------------------
    bh2 = float(np.asarray(inp["bh2"]).reshape(-1)[0])
    y_full = np.concatenate([np.asarray(outs[c]["yout"]) for c in range(NCORES)],
                            axis=1)  # [6, B]
    hp = 1.0 / (1.0 + np.exp(-(y_full.astype(np.float64) + bh2)))
    hmeans = hp.mean(axis=1)
    T = STEPS
    for t in range(STEPS):
        if hmeans[t] > 0.5:
            T = t + 1
            break

    mask = np.empty((B, DC), np.float32)
    epsilon = np.empty((B, DC), np.float32)
    z_state = np.empty((B, DC), np.float32)
    for c in range(NCORES):
        rows = slice(c * BSH, (c + 1) * BSH)
        mask[rows] = np.asarray(outs[c]["maskT"]).T
        epsilon[rows] = np.asarray(outs[c]["epsT"]).T
        z_state[rows] = np.asarray(outs[c]["zallT"])[T - 1].T.astype(np.float32)

    return z_state, mask, epsilon, np.int32(T)


# revision 29
# speedup vs baseline: 1.0955x; 1.0955x over previous
"""Trainium2 Bass kernel for nn_AdaptiveInterventionEngine.

Data-parallel over batch across 8 NeuronCores. Feature-major (transposed)
activation layout on device so every weight matrix is used as lhsT exactly
as stored. Encoder matmuls bf16 (fp32 PSUM accumulate), LayerNorm stats in
fp32 via ones-vector matmuls on the PE, recurrence in fp32 via float32r
matmuls. The per-step halt decision is decoupled: the device computes the
full 6-step trajectory plus per-element halter logits; the host reduces the
logit means, derives steps_used and selects the matching z state.
"""

import math

import numpy as np
import ml_dtypes

import concourse.bass as bass
import concourse.mybir as mybir
import concourse.tile as tile
from concourse import bacc
from concourse.bass import ts
from concourse.bass_utils import run_bass_kernel_spmd

BF16NP = ml_dtypes.bfloat16
F32 = mybir.dt.float32
F32R = mybir.dt.float32r
BF16 = mybir.dt.bfloat16
AF = mybir.ActivationFunctionType
OP = mybir.AluOpType

NCORES = 8
B = 32768
BSH = B // NCORES          # 4096 batch rows per core
DC = 512                   # causal dim
DA = 1024                  # action dim
HID = 2048                 # hidden dim
ENC_D = DC // 4            # 128
STEPS = 6
NT = 256                   # batch columns per tile
NTILES = BSH // NT         # 16
KA = DA // 128             # 8  action k-chunks
KH = HID // 128            # 16 hidden k-chunks
KC = DC // 128             # 4  causal k-chunks

TRACE = False
LAST_EXEC_NS = None
LAST_RESULTS = None


def _sin_enc(step_f, d):
    div = np.exp(np.arange(0, d, 2, dtype=np.float64) * (-(math.log(10000.0) / d)))
    pe = np.zeros((d,), dtype=np.float64)
    pe[0::2] = np.sin(step_f * div[: (d + 1) // 2])
    pe[1::2] = np.cos(step_f * div[: d // 2])
    return pe


def _pcol(v):
    """[n*128] vector -> [128, n] (partition, m-chunk) layout, f32."""
    v = np.asarray(v, np.float32)
    n = v.shape[0] // 128
    return np.ascontiguousarray(v.reshape(n, 128).T)


def build_nc():
    nc = bacc.Bacc(
        "TRN2", target_bir_lowering=False, debug=False, num_devices=NCORES
    )

    def din(name, shape, dt):
        return nc.dram_tensor(name, shape, dt, kind="ExternalInput").ap()

    def dout(name, shape, dt):
        return nc.dram_tensor(name, shape, dt, kind="ExternalOutput").ap()

    actT_d = din("actionT", [DA, BSH], BF16)
    zT_d = din("zT", [DC, BSH], F32)
    zbf_d = din("zTbf", [DC, BSH], BF16)
    w1_d = din("w1", [DA, HID], BF16)
    w2_d = din("w2", [HID, HID], BF16)
    wmv_d = din("wmv", [HID, 2 * DC], BF16)
    adj_d = din("adj", [DC, DC], BF16)
    wh1_d = din("wh1a", [DC, DC], BF16)
    wh2_d = din("wh2", [DC, 1], BF16)
    b1g_d = din("b1g", [128, 3 * KH], F32)    # b1 | g1 | be1
    b2g_d = din("b2g", [128, 3 * KH], F32)    # b2 | g2 | be2
    bmv_d = din("bmv", [128, 2 * KC], F32)    # -bm | bv
    cst_d = din("cst", [128, STEPS * KC], F32)  # halter per-step bias

    maskT_o = dout("maskT", [DC, BSH], F32)
    epsT_o = dout("epsT", [DC, BSH], F32)
    zallT_o = dout("zallT", [STEPS, DC, BSH], BF16)
    y_o = dout("yout", [STEPS, BSH], F32)

    with tile.TileContext(nc) as tc:
        build_kernel(
            nc, tc,
            actT_d, zT_d, zbf_d, w1_d, w2_d, wmv_d, adj_d, wh1_d, wh2_d,
            b1g_d, b2g_d, bmv_d, cst_d,
            maskT_o, epsT_o, zallT_o, y_o,
        )
    nc.compile()
    return nc


def build_kernel(nc, tc, actT_d, zT_d, zbf_d, w1_d, w2_d, wmv_d, adj_d, wh1_d, wh2_d,
                 b1g_d, b2g_d, bmv_d, cst_d, maskT_o, epsT_o, zallT_o, y_o):
    from contextlib import ExitStack

    ctx = ExitStack()
    with ctx:
        singles = ctx.enter_context(tc.tile_pool(name="singles", bufs=1))
        w2p = ctx.enter_context(tc.tile_pool(name="w2p", bufs=3))
        actp = ctx.enter_context(tc.tile_pool(name="actp", bufs=2))
        tp = ctx.enter_context(tc.tile_pool(name="tp", bufs=2))
        h1p = ctx.enter_context(tc.tile_pool(name="h1p", bufs=2))
        h2p = ctx.enter_context(tc.tile_pool(name="h2p", bufs=2))
        sqp = ctx.enter_context(tc.tile_pool(name="sqp", bufs=2))
        stp = ctx.enter_context(tc.tile_pool(name="stp", bufs=6))
        bcp = ctx.enter_context(tc.tile_pool(name="bcp", bufs=2))
        nscp = ctx.enter_context(tc.tile_pool(name="nscp", bufs=3))
        zinp = ctx.enter_context(tc.tile_pool(name="zinp", bufs=2))
        zbfp = ctx.enter_context(tc.tile_pool(name="zbfp", bufs=1))
        m1p = ctx.enter_context(tc.tile_pool(name="m1p", bufs=2))
        valp = ctx.enter_context(tc.tile_pool(name="valp", bufs=1))
        epsp = ctx.enter_context(tc.tile_pool(name="epsp", bufs=1))
        maskp = ctx.enter_context(tc.tile_pool(name="maskp", bufs=1))
        fscp = ctx.enter_context(tc.tile_pool(name="fscp", bufs=4))
        cp = ctx.enter_context(tc.tile_pool(name="cp", bufs=2))
        zp = ctx.enter_context(tc.tile_pool(name="zp", bufs=3))
        dltp = ctx.enter_context(tc.tile_pool(name="dltp", bufs=1))
        gp = ctx.enter_context(tc.tile_pool(name="gp", bufs=1))
        yp = ctx.enter_context(tc.tile_pool(name="yp", bufs=1))
        mmps = ctx.enter_context(tc.tile_pool(name="mmps", bufs=5, space="PSUM"))
        auxps = ctx.enter_context(tc.tile_pool(name="auxps", bufs=3, space="PSUM"))

        # ---- resident weights -------------------------------------------
        w1_s = singles.tile([128, KA, HID], BF16, name="w1_s")
        nc.sync.dma_start(out=w1_s, in_=w1_d.rearrange("(k p) m -> p k m", p=128))
        wmv_s = singles.tile([128, KH, 2 * DC], BF16, name="wmv_s")
        nc.sync.dma_start(out=wmv_s, in_=wmv_d.rearrange("(k p) m -> p k m", p=128))
        adj_s = singles.tile([128, KC, DC], BF16, name="adj_s")
        nc.sync.dma_start(out=adj_s, in_=adj_d.rearrange("(k p) m -> p k m", p=128))
        wh1_s = singles.tile([128, KC, DC], BF16, name="wh1_s")
        nc.sync.dma_start(out=wh1_s, in_=wh1_d.rearrange("(k p) m -> p k m", p=128))
        wh2_s = singles.tile([128, KC], BF16, name="wh2_s")
        nc.sync.dma_start(out=wh2_s, in_=wh2_d.rearrange("(k p) o -> p (k o)", p=128))
        b1g_s = singles.tile([128, 3 * KH], F32, name="b1g_s")
        nc.sync.dma_start(out=b1g_s, in_=b1g_d)
        b2g_s = singles.tile([128, 3 * KH], F32, name="b2g_s")
        nc.sync.dma_start(out=b2g_s, in_=b2g_d)
        bmv_s = singles.tile([128, 2 * KC], F32, name="bmv_s")
        nc.sync.dma_start(out=bmv_s, in_=bmv_d)
        cst_s = singles.tile([128, STEPS * KC], F32, name="cst_s")
        nc.sync.dma_start(out=cst_s, in_=cst_d)
        ones_bf = singles.tile([128, 1], BF16, name="ones_bf")
        nc.vector.memset(ones_bf, 1.0)
        ones_r = singles.tile([1, 128], BF16, name="ones_r")
        nc.vector.memset(ones_r, 1.0)
        epsb = singles.tile([1, 1], F32, name="epsb")
        nc.vector.memset(epsb, 1e-5)

        actT_r = actT_d.rearrange("(k p) n -> p k n", p=128)
        zT_r = zT_d.rearrange("(k p) n -> p k n", p=128)
        zbf_r = zbf_d.rearrange("(k p) n -> p k n", p=128)
        maskT_r = maskT_o.rearrange("(m p) n -> p m n", p=128)
        epsT_r = epsT_o.rearrange("(m p) n -> p m n", p=128)
        zallT_r = zallT_o.rearrange("t (m p) n -> t p m n", p=128)

        def bcast2(ap2d, n):
            return bass.AP(tensor=ap2d.tensor, offset=ap2d.offset,
                           ap=[ap2d.ap[0], [0, n], ap2d.ap[1]])

        def mm_quarters(M, K, lhsT_fn, rhs_fn, drain_fn, q=4):
            """out_mtile[m] = sum_k lhsT(k,m).T @ rhs(k); M,K in 128-tiles."""
            for q0 in range(0, M, q):
                nq = min(q, M - q0)
                pss = []
                for mi in range(nq):
                    ps = mmps.tile([128, NT], F32, tag="mm", name="ps")
                    pss.append(ps)
                for k in range(K):
                    for mi in range(nq):
                        nc.tensor.matmul(
                            pss[mi], lhsT_fn(k, q0 + mi), rhs_fn(k),
                            start=(k == 0), stop=(k == K - 1),
                        )
                for mi in range(nq):
                    drain_fn(q0 + mi, pss[mi])

        def layer_stats_norm(t_s, h_s, ktiles, gvec_off, bias_off, bg_s):
            """LN over partition-axis features + gelu, t_s -> h_s (bf16)."""
            sum_ps = auxps.tile([1, NT], F32, tag="aux", name="sum_ps")
            sq_ps = auxps.tile([1, NT], F32, tag="aux", name="sq_ps")
            for k in range(ktiles):
                sl = t_s[:, ts(k, NT)]
                sq_t = sqp.tile([128, NT], BF16, tag="sq", name="sq_t")
                nc.gpsimd.tensor_tensor(sq_t, sl, sl, op=OP.mult)
                nc.tensor.matmul(sum_ps, ones_bf, sl,
                                 start=(k == 0), stop=(k == ktiles - 1))
                nc.tensor.matmul(sq_ps, ones_bf, sq_t,
                                 start=(k == 0), stop=(k == ktiles - 1))
            inv_n = 1.0 / (ktiles * 128)
            mean_s = stp.tile([1, NT], F32, tag="st", name="mean_s")
            nc.scalar.activation(mean_s, sum_ps, AF.Copy, scale=inv_n)
            ex2_s = stp.tile([1, NT], F32, tag="st", name="ex2_s")
            nc.scalar.activation(ex2_s, sq_ps, AF.Identity, bias=epsb, scale=inv_n)
            msq_s = stp.tile([1, NT], F32, tag="st", name="msq_s")
            nc.vector.tensor_tensor(msq_s, mean_s, mean_s, op=OP.mult)
            vpe_s = stp.tile([1, NT], F32, tag="st", name="vpe_s")
            nc.vector.tensor_tensor(vpe_s, ex2_s, msq_s, op=OP.subtract)
            sd_s = stp.tile([1, NT], F32, tag="st", name="sd_s")
            nc.scalar.activation(sd_s, vpe_s, AF.Sqrt)
            rstd_s = stp.tile([1, NT], BF16, tag="st", name="rstd_s")
            nmr_s = stp.tile([1, NT], BF16, tag="st", name="nmr_s")
            with nc.allow_low_precision(reason="LN scale vectors feed bf16 bcast matmul"):
                nc.vector.reciprocal(rstd_s, sd_s)
                nc.vector.scalar_tensor_tensor(
                    nmr_s, mean_s, -1.0, rstd_s, op0=OP.mult, op1=OP.mult)

            bc_ps = auxps.tile([128, NT], F32, tag="aux", name="bc_ps")
            nc.tensor.matmul(bc_ps, ones_r, rstd_s, start=True, stop=True)
            rstd_b = bcp.tile([128, NT], F32, tag="bc", name="rstd_b")
            nc.scalar.activation(rstd_b, bc_ps, AF.Copy)
            bc2_ps = auxps.tile([128, NT], F32, tag="aux", name="bc2_ps")
            nc.tensor.matmul(bc2_ps, ones_r, nmr_s, start=True, stop=True)
            nmr_b = bcp.tile([128, NT], F32, tag="bc", name="nmr_b")
            nc.scalar.activation(nmr_b, bc2_ps, AF.Copy)

            for k in range(ktiles):
                x1_t = nscp.tile([128, NT], F32, tag="nsc", name="x1_t")
                nc.vector.tensor_tensor(x1_t, t_s[:, ts(k, NT)], rstd_b, op=OP.mult)
                x2_t = nscp.tile([128, NT], F32, tag="nsc", name="x2_t")
                nc.vector.tensor_tensor(x2_t, x1_t, nmr_b, op=OP.add)
                nc.scalar.activation(
                    h_s[:, ts(k, NT)], x2_t, AF.Gelu,
                    bias=bg_s[:, bias_off + k:bias_off + k + 1],
                    scale=bg_s[:, gvec_off + k:gvec_off + k + 1],
                )

        w2_r = w2_d.rearrange("(k p) m -> p k m", p=128)

        def tile_body(j):
            cols = slice(j * NT, (j + 1) * NT)
            a_s = actp.tile([128, KA * NT], BF16, tag="act", name="a_s")
            nc.sync.dma_start(
                out=a_s.rearrange("p (k n) -> p k n", k=KA),
                in_=actT_r[:, :, cols],
            )
            zin_s = zinp.tile([128, KC * NT], F32, tag="zin", name="zin_s")
            nc.sync.dma_start(
                out=zin_s.rearrange("p (k n) -> p k n", k=KC),
                in_=zT_r[:, :, cols],
            )
            zbf_s = zbfp.tile([128, KC * NT], BF16, tag="zbf", name="zbf_s")
            nc.sync.dma_start(
                out=zbf_s.rearrange("p (k n) -> p k n", k=KC),
                in_=zbf_r[:, :, cols],
            )

            t1_s = tp.tile([128, KH * NT], BF16, tag="t", name="t1_s")

            def l1_drain(m, ps):
                nc.scalar.activation(t1_s[:, ts(m, NT)], ps, AF.Identity,
                                     bias=b1g_s[:, m:m + 1])

            mm_quarters(
                KH, KA,
                lambda k, m: w1_s[:, k, ts(m, 128)],
                lambda k: a_s[:, ts(k, NT)],
                l1_drain,
            )
            h1_s = h1p.tile([128, KH * NT], BF16, tag="h1", name="h1_s")
            layer_stats_norm(t1_s, h1_s, KH, KH, 2 * KH, b1g_s)

            t2_s = tp.tile([128, KH * NT], BF16, tag="t", name="t2_s")
            for q0 in range(0, KH, 4):
                pss = []
                for mi in range(4):
                    ps = mmps.tile([128, NT], F32, tag="mm", name="ps")
                    pss.append(ps)
                for kg in range(0, KH, 4):
                    wt = w2p.tile([128, 4, 4 * 128], BF16, tag="w2", name="wt")
                    nc.sync.dma_start(
                        out=wt,
                        in_=w2_r[:, kg:kg + 4, q0 * 128:(q0 + 4) * 128],
                    )
                    for dk in range(4):
                        k = kg + dk
                        for mi in range(4):
                            nc.tensor.matmul(
                                pss[mi], wt[:, dk, ts(mi, 128)],
                                h1_s[:, ts(k, NT)],
                                start=(k == 0), stop=(k == KH - 1),
                            )
                for mi in range(4):
                    m = q0 + mi
                    nc.scalar.activation(t2_s[:, ts(m, NT)], pss[mi], AF.Identity,
                                         bias=b2g_s[:, m:m + 1])
            h2_s = h2p.tile([128, KH * NT], BF16, tag="h2", name="h2_s")
            layer_stats_norm(t2_s, h2_s, KH, KH, 2 * KH, b2g_s)

            m1_s = m1p.tile([128, KC * NT], F32, tag="m1", name="m1_s")
            val_s = valp.tile([128, KC * NT], F32, tag="val", name="val_s")
            mask_s = maskp.tile([128, KC * NT], F32, tag="mask", name="mask_s")

            def mv_drain(m, ps):
                if m < KC:
                    nc.scalar.activation(m1_s[:, ts(m, NT)], ps, AF.Sigmoid,
                                         bias=bmv_s[:, m:m + 1], scale=-1.0)
                    nc.vector.tensor_scalar(
                        mask_s[:, ts(m, NT)], m1_s[:, ts(m, NT)],
                        -1.0, 1.0, op0=OP.mult, op1=OP.add)
                else:
                    mm = m - KC
                    nc.scalar.activation(val_s[:, ts(mm, NT)], ps, AF.Identity,
                                         bias=bmv_s[:, KC + mm:KC + mm + 1])

            mm_quarters(
                2 * KC, KH,
                lambda k, m: wmv_s[:, k, ts(m, 128)],
                lambda k: h2_s[:, ts(k, NT)],
                mv_drain,
            )
            nc.sync.dma_start(
                out=maskT_r[:, :, cols],
                in_=mask_s.rearrange("p (m n) -> p m n", m=KC),
            )

            eps_s = epsp.tile([128, KC * NT], F32, tag="eps", name="eps_s")

            def eps_drain(m, ps):
                nc.vector.tensor_tensor(
                    eps_s[:, ts(m, NT)], zin_s[:, ts(m, NT)], ps, op=OP.subtract)

            mm_quarters(
                KC, KC,
                lambda k, m: adj_s[:, k, ts(m, 128)],
                lambda k: zbf_s[:, ts(k, NT)],
                eps_drain,
            )
            nc.sync.dma_start(
                out=epsT_r[:, :, cols],
                in_=eps_s.rearrange("p (m n) -> p m n", m=KC),
            )

            z0_s = zp.tile([128, KC * NT], BF16, tag="z", name="z0_s")
            c_s = cp.tile([128, KC * NT], F32, tag="c", name="c_s")
            for m in range(KC):
                sl = ts(m, NT)
                d1 = fscp.tile([128, NT], F32, tag="fsc2", name="d1")
                nc.gpsimd.tensor_tensor(d1, zin_s[:, sl], val_s[:, sl],
                                        op=OP.subtract)
                p1 = fscp.tile([128, NT], F32, tag="fsc2", name="p1")
                nc.vector.tensor_tensor(p1, d1, m1_s[:, sl], op=OP.mult)
                z0f = fscp.tile([128, NT], F32, tag="fsc2", name="z0f")
                nc.vector.tensor_tensor(z0f, val_s[:, sl], p1, op=OP.add)
                nc.gpsimd.tensor_copy(z0_s[:, sl], z0f)
                e2 = fscp.tile([128, NT], F32, tag="fsc2", name="e2")
                nc.gpsimd.tensor_tensor(e2, eps_s[:, sl], z0f, op=OP.subtract)
                p2 = fscp.tile([128, NT], F32, tag="fsc2", name="p2")
                nc.vector.tensor_tensor(p2, e2, m1_s[:, sl], op=OP.mult)
                nc.vector.tensor_tensor(c_s[:, sl], z0f, p2, op=OP.add)
            return dict(cols=cols, m1_s=m1_s, c_s=c_s, z_cur=z0_s)

        def tile_step(st, t):
            cols = st["cols"]
            m1_s = st["m1_s"]
            c_s = st["c_s"]
            z_cur = st["z_cur"]
            z_new = zp.tile([128, KC * NT], BF16, tag="z", name="z_new")
            delta_s = dltp.tile([128, KC * NT], BF16, tag="dlt", name="delta_s")

            def z_drain(m, ps):
                sl = ts(m, NT)
                p1 = fscp.tile([128, NT], F32, tag="fsc2", name="zp1")
                nc.vector.tensor_tensor(p1, ps, m1_s[:, sl], op=OP.mult)
                nc.vector.tensor_tensor(z_new[:, sl], c_s[:, sl], p1, op=OP.add)
                ds_ = fscp.tile([128, NT], BF16, tag="dsc", name="ds_")
                nc.gpsimd.tensor_tensor(ds_, z_new[:, sl], z_cur[:, sl],
                                        op=OP.subtract)
                nc.scalar.activation(delta_s[:, sl], ds_, AF.Abs)

            mm_quarters(
                KC, KC,
                lambda k, m: adj_s[:, k, ts(m, 128)],
                lambda k: z_cur[:, ts(k, NT)],
                z_drain,
            )
            nc.sync.dma_start(
                out=zallT_r[t, :, :, cols],
                in_=z_new.rearrange("p (m n) -> p m n", m=KC),
            )

            g_s = gp.tile([128, KC * NT], BF16, tag="g", name="g_s")

            def g_drain(m, ps):
                nc.scalar.activation(g_s[:, ts(m, NT)], ps, AF.Gelu,
                                     bias=cst_s[:, t * KC + m:t * KC + m + 1])

            mm_quarters(
                KC, KC,
                lambda k, m: wh1_s[:, k, ts(m, 128)],
                lambda k: delta_s[:, ts(k, NT)],
                g_drain,
            )

            y_ps = auxps.tile([1, NT], F32, tag="aux", name="y_ps")
            for k in range(KC):
                nc.tensor.matmul(
                    y_ps, wh2_s[:, k:k + 1], g_s[:, ts(k, NT)],
                    start=(k == 0), stop=(k == KC - 1),
                )
            y_row = yp.tile([1, NT], F32, tag="y", name="y_row")
            nc.scalar.activation(y_row, y_ps, AF.Copy)
            nc.sync.dma_start(out=y_o[t:t + 1, cols], in_=y_row)
            st["z_cur"] = z_new

        def preload(j):
            cols = slice(j * NT, (j + 1) * NT)
            a_s = actp.tile([128, KA * NT], BF16, tag="act", name="a_s")
            nc.sync.dma_start(
                out=a_s.rearrange("p (k n) -> p k n", k=KA),
                in_=actT_r[:, :, cols],
            )
            zin_s = zinp.tile([128, KC * NT], F32, tag="zin", name="zin_s")
            nc.sync.dma_start(
                out=zin_s.rearrange("p (k n) -> p k n", k=KC),
                in_=zT_r[:, :, cols],
            )
            return dict(cols=cols, a_s=a_s, zin_s=zin_s)

        for j in range(NTILES):
            sj = tile_body(j)
            for t in range(STEPS):
                tile_step(sj, t)


_built = None


def _get_nc():
    global _built
    if _built is None:
        _built = build_nc()
    return _built


def kernel(**inputs):
    global LAST_EXEC_NS, LAST_RESULTS
    inp = {k: np.asarray(v) for k, v in inputs.items()}

    w1 = inp["w1"].astype(BF16NP)
    w2 = inp["w2"].astype(BF16NP)
    wmv = np.concatenate([inp["wm"], inp["wv"]], axis=1).astype(BF16NP)
    adj = inp["adjacency"].astype(BF16NP)
    wh1a = inp["wh1"][:DC].astype(BF16NP)
    wh2 = inp["wh2"].astype(BF16NP)
    b1g = np.concatenate(
        [_pcol(inp["b1"]), _pcol(inp["g1"]), _pcol(inp["be1"])], axis=1)
    b2g = np.concatenate(
        [_pcol(inp["b2"]), _pcol(inp["g2"]), _pcol(inp["be2"])], axis=1)
    bmv = np.concatenate([_pcol(-inp["bm"]), _pcol(inp["bv"])], axis=1)
    cst_cols = []
    wh1b = inp["wh1"][DC:].astype(np.float64)
    bh1 = inp["bh1"].astype(np.float64)
    for t in range(STEPS):
        enc = _sin_enc(float(t), ENC_D)
        cst_cols.append(_pcol((enc @ wh1b + bh1).astype(np.float32)))
    cst = np.concatenate(cst_cols, axis=1)

    shared = dict(w1=w1, w2=w2, wmv=wmv, adj=adj, wh1a=wh1a, wh2=wh2,
                  b1g=b1g, b2g=b2g, bmv=bmv, cst=cst)

    in_maps = []
    for c in range(NCORES):
        rows = slice(c * BSH, (c + 1) * BSH)
        m = dict(shared)
        m["actionT"] = np.ascontiguousarray(inp["action"][rows].T).astype(BF16NP)
        zt = np.ascontiguousarray(inp["z"][rows].T.astype(np.float32))
        m["zT"] = zt
        m["zTbf"] = zt.astype(BF16NP)
        in_maps.append(m)

    nc = _get_nc()
    res = run_bass_kernel_spmd(
        nc, in_maps, core_ids=list(range(NCORES)), trace=TRACE
    )
    LAST_EXEC_NS = res.exec_time_ns
    LAST_RESULTS = res
    outs = res.results

    # ---- host-side unshard + halt decision --------------------------------
    bh2 = float(np.asarray(inp["bh2"]).reshape(-1)[0])
    y_full = np.concatenate([np.asarray(outs[c]["yout"]) for c in range(NCORES)],
                            axis=1)  # [6, B]
    hp = 1.0 / (1.0 + np.exp(-(y_full.astype(np.float64) + bh2)))
    hmeans = hp.mean(axis=1)
    T = STEPS
    for t in range(STEPS):
        if hmeans[t] > 0.5:
            T = t + 1
            break

    mask = np.empty((B, DC), np.float32)
    epsilon = np.empty((B, DC), np.float32)
    z_state = np.empty((B, DC), np.float32)
    for c in range(NCORES):
        rows = slice(c * BSH, (c + 1) * BSH)
        mask[rows] = np.asarray(outs[c]["maskT"]).T
        epsilon[rows] = np.asarray(outs[c]["epsT"]).T
        z_state[rows] = np.asarray(outs[c]["zallT"])[T - 1].T.astype(np.float32)

    return z_state, mask, epsilon, np.int32(T)


# revision 31
# speedup vs baseline: 1.2095x; 1.1041x over previous
"""Trainium2 Bass kernel for nn_AdaptiveInterventionEngine.

Data-parallel over batch across 8 NeuronCores. Feature-major (transposed)
activation layout on device so every weight matrix is used as lhsT exactly
as stored. Encoder matmuls bf16 (fp32 PSUM accumulate), LayerNorm stats in
fp32 via ones-vector matmuls on the PE, recurrence in fp32 via float32r
matmuls. The per-step halt decision is decoupled: the device computes the
full 6-step trajectory plus per-element halter logits; the host reduces the
logit means, derives steps_used and selects the matching z state.
"""

import math

import numpy as np
import ml_dtypes

import concourse.bass as bass
import concourse.mybir as mybir
import concourse.tile as tile
from concourse import bacc
from concourse.bass import ts
from concourse.bass_utils import run_bass_kernel_spmd

BF16NP = ml_dtypes.bfloat16
F32 = mybir.dt.float32
F32R = mybir.dt.float32r
BF16 = mybir.dt.bfloat16
AF = mybir.ActivationFunctionType
OP = mybir.AluOpType

NCORES = 8
B = 32768
BSH = B // NCORES          # 4096 batch rows per core
DC = 512                   # causal dim
DA = 1024                  # action dim
HID = 2048                 # hidden dim
ENC_D = DC // 4            # 128
STEPS = 6
NT = 256                   # batch columns per tile
NTILES = BSH // NT         # 16
KA = DA // 128             # 8  action k-chunks
KH = HID // 128            # 16 hidden k-chunks
KC = DC // 128             # 4  causal k-chunks

TRACE = False
LAST_EXEC_NS = None
LAST_RESULTS = None


def _sin_enc(step_f, d):
    div = np.exp(np.arange(0, d, 2, dtype=np.float64) * (-(math.log(10000.0) / d)))
    pe = np.zeros((d,), dtype=np.float64)
    pe[0::2] = np.sin(step_f * div[: (d + 1) // 2])
    pe[1::2] = np.cos(step_f * div[: d // 2])
    return pe


def _pcol(v):
    """[n*128] vector -> [128, n] (partition, m-chunk) layout, f32."""
    v = np.asarray(v, np.float32)
    n = v.shape[0] // 128
    return np.ascontiguousarray(v.reshape(n, 128).T)


def build_nc():
    nc = bacc.Bacc(
        "TRN2", target_bir_lowering=False, debug=False, num_devices=NCORES
    )

    def din(name, shape, dt):
        return nc.dram_tensor(name, shape, dt, kind="ExternalInput").ap()

    def dout(name, shape, dt):
        return nc.dram_tensor(name, shape, dt, kind="ExternalOutput").ap()

    actT_d = din("actionT", [DA, BSH], BF16)
    zT_d = din("zT", [DC, BSH], F32)
    zbf_d = din("zTbf", [DC, BSH], BF16)
    w1_d = din("w1", [DA, HID], BF16)
    w2_d = din("w2", [HID, HID], BF16)
    wmv_d = din("wmv", [HID, 2 * DC], BF16)
    adj_d = din("adj", [DC, DC], BF16)
    wh1_d = din("wh1a", [DC, DC], BF16)
    wh2_d = din("wh2", [DC, 1], BF16)
    b1g_d = din("b1g", [128, 3 * KH], F32)    # b1 | g1 | be1
    b2g_d = din("b2g", [128, 3 * KH], F32)    # b2 | g2 | be2
    bmv_d = din("bmv", [128, 2 * KC], F32)    # -bm | bv
    cst_d = din("cst", [128, STEPS * KC], F32)  # halter per-step bias

    maskT_o = dout("maskT", [DC, BSH], F32)
    epsT_o = dout("epsT", [DC, BSH], F32)
    zallT_o = dout("zallT", [STEPS, DC, BSH], BF16)
    y_o = dout("yout", [STEPS, BSH], F32)

    with tile.TileContext(nc) as tc:
        build_kernel(
            nc, tc,
            actT_d, zT_d, zbf_d, w1_d, w2_d, wmv_d, adj_d, wh1_d, wh2_d,
            b1g_d, b2g_d, bmv_d, cst_d,
            maskT_o, epsT_o, zallT_o, y_o,
        )
    nc.compile()
    return nc


def build_kernel(nc, tc, actT_d, zT_d, zbf_d, w1_d, w2_d, wmv_d, adj_d, wh1_d, wh2_d,
                 b1g_d, b2g_d, bmv_d, cst_d, maskT_o, epsT_o, zallT_o, y_o):
    from contextlib import ExitStack

    ctx = ExitStack()
    with ctx:
        singles = ctx.enter_context(tc.tile_pool(name="singles", bufs=1))
        w2p = ctx.enter_context(tc.tile_pool(name="w2p", bufs=3))
        actp = ctx.enter_context(tc.tile_pool(name="actp", bufs=2))
        tp = ctx.enter_context(tc.tile_pool(name="tp", bufs=2))
        h1p = ctx.enter_context(tc.tile_pool(name="h1p", bufs=2))
        h2p = ctx.enter_context(tc.tile_pool(name="h2p", bufs=2))
        sqp = ctx.enter_context(tc.tile_pool(name="sqp", bufs=2))
        stp = ctx.enter_context(tc.tile_pool(name="stp", bufs=6))
        bcp = ctx.enter_context(tc.tile_pool(name="bcp", bufs=2))
        nscp = ctx.enter_context(tc.tile_pool(name="nscp", bufs=3))
        zinp = ctx.enter_context(tc.tile_pool(name="zinp", bufs=2))
        zbfp = ctx.enter_context(tc.tile_pool(name="zbfp", bufs=1))
        m1p = ctx.enter_context(tc.tile_pool(name="m1p", bufs=2))
        valp = ctx.enter_context(tc.tile_pool(name="valp", bufs=1))
        epsp = ctx.enter_context(tc.tile_pool(name="epsp", bufs=1))
        maskp = ctx.enter_context(tc.tile_pool(name="maskp", bufs=1))
        fscp = ctx.enter_context(tc.tile_pool(name="fscp", bufs=4))
        cp = ctx.enter_context(tc.tile_pool(name="cp", bufs=2))
        zp = ctx.enter_context(tc.tile_pool(name="zp", bufs=3))
        dltp = ctx.enter_context(tc.tile_pool(name="dltp", bufs=1))
        gp = ctx.enter_context(tc.tile_pool(name="gp", bufs=1))
        yp = ctx.enter_context(tc.tile_pool(name="yp", bufs=1))
        mmps = ctx.enter_context(tc.tile_pool(name="mmps", bufs=5, space="PSUM"))
        auxps = ctx.enter_context(tc.tile_pool(name="auxps", bufs=3, space="PSUM"))

        # ---- resident weights -------------------------------------------
        w1_s = singles.tile([128, KA, HID], BF16, name="w1_s")
        nc.sync.dma_start(out=w1_s, in_=w1_d.rearrange("(k p) m -> p k m", p=128))
        wmv_s = singles.tile([128, KH, 2 * DC], BF16, name="wmv_s")
        nc.sync.dma_start(out=wmv_s, in_=wmv_d.rearrange("(k p) m -> p k m", p=128))
        adj_s = singles.tile([128, KC, DC], BF16, name="adj_s")
        nc.sync.dma_start(out=adj_s, in_=adj_d.rearrange("(k p) m -> p k m", p=128))
        wh1_s = singles.tile([128, KC, DC], BF16, name="wh1_s")
        nc.sync.dma_start(out=wh1_s, in_=wh1_d.rearrange("(k p) m -> p k m", p=128))
        wh2_s = singles.tile([128, KC], BF16, name="wh2_s")
        nc.sync.dma_start(out=wh2_s, in_=wh2_d.rearrange("(k p) o -> p (k o)", p=128))
        b1g_s = singles.tile([128, 3 * KH], F32, name="b1g_s")
        nc.sync.dma_start(out=b1g_s, in_=b1g_d)
        b2g_s = singles.tile([128, 3 * KH], F32, name="b2g_s")
        nc.sync.dma_start(out=b2g_s, in_=b2g_d)
        bmv_s = singles.tile([128, 2 * KC], F32, name="bmv_s")
        nc.sync.dma_start(out=bmv_s, in_=bmv_d)
        cst_s = singles.tile([128, STEPS * KC], F32, name="cst_s")
        nc.sync.dma_start(out=cst_s, in_=cst_d)
        ones_bf = singles.tile([128, 1], BF16, name="ones_bf")
        nc.vector.memset(ones_bf, 1.0)
        ones_r = singles.tile([1, 128], BF16, name="ones_r")
        nc.vector.memset(ones_r, 1.0)
        epsb = singles.tile([1, 1], F32, name="epsb")
        nc.vector.memset(epsb, 1e-5)

        actT_r = actT_d.rearrange("(k p) n -> p k n", p=128)
        zT_r = zT_d.rearrange("(k p) n -> p k n", p=128)
        zbf_r = zbf_d.rearrange("(k p) n -> p k n", p=128)
        maskT_r = maskT_o.rearrange("(m p) n -> p m n", p=128)
        epsT_r = epsT_o.rearrange("(m p) n -> p m n", p=128)
        zallT_r = zallT_o.rearrange("t (m p) n -> t p m n", p=128)

        def bcast2(ap2d, n):
            return bass.AP(tensor=ap2d.tensor, offset=ap2d.offset,
                           ap=[ap2d.ap[0], [0, n], ap2d.ap[1]])

        def mm_quarters(M, K, lhsT_fn, rhs_fn, drain_fn, q=4):
            """out_mtile[m] = sum_k lhsT(k,m).T @ rhs(k); M,K in 128-tiles."""
            for q0 in range(0, M, q):
                nq = min(q, M - q0)
                pss = []
                for mi in range(nq):
                    ps = mmps.tile([128, NT], F32, tag="mm", name="ps")
                    pss.append(ps)
                for k in range(K):
                    for mi in range(nq):
                        nc.tensor.matmul(
                            pss[mi], lhsT_fn(k, q0 + mi), rhs_fn(k),
                            start=(k == 0), stop=(k == K - 1),
                        )
                for mi in range(nq):
                    drain_fn(q0 + mi, pss[mi])

        w2_r = w2_d.rearrange("(k p) m -> p k m", p=128)

        def preload(j):
            cols = slice(j * NT, (j + 1) * NT)
            a_s = actp.tile([128, KA * NT], BF16, tag="act", name="a_s")
            nc.sync.dma_start(
                out=a_s.rearrange("p (k n) -> p k n", k=KA),
                in_=actT_r[:, :, cols],
            )
            zin_s = zinp.tile([128, KC * NT], F32, tag="zin", name="zin_s")
            nc.sync.dma_start(
                out=zin_s.rearrange("p (k n) -> p k n", k=KC),
                in_=zT_r[:, :, cols],
            )
            zbf_s = zbfp.tile([128, KC * NT], BF16, tag="zbf", name="zbf_s")
            nc.sync.dma_start(
                out=zbf_s.rearrange("p (k n) -> p k n", k=KC),
                in_=zbf_r[:, :, cols],
            )
            return dict(cols=cols, a_s=a_s, zin_s=zin_s, zbf_s=zbf_s)

        def l1_chunks(p):
            """Closures emitting L1 quarters + stats for one tile."""
            a_s = p["a_s"]
            t1_s = tp.tile([128, KH * NT], BF16, tag="t", name="t1_s")
            p["t1_s"] = t1_s

            def l1_drain(m, ps):
                nc.scalar.activation(t1_s[:, ts(m, NT)], ps, AF.Identity,
                                     bias=b1g_s[:, m:m + 1])

            def quarter(q0):
                def run():
                    pss = []
                    for mi in range(4):
                        psq = mmps.tile([128, NT], F32, tag="mm", name="ps")
                        pss.append(psq)
                    for k in range(KA):
                        for mi in range(4):
                            nc.tensor.matmul(
                                pss[mi], w1_s[:, k, ts(q0 + mi, 128)],
                                a_s[:, ts(k, NT)],
                                start=(k == 0), stop=(k == KA - 1),
                            )
                    for mi in range(4):
                        l1_drain(q0 + mi, pss[mi])
                return run

            def stats():
                sum_ps = auxps.tile([1, NT], F32, tag="aux", name="sum_ps")
                sq_ps = auxps.tile([1, NT], F32, tag="aux", name="sq_ps")
                for k in range(KH):
                    sl = t1_s[:, ts(k, NT)]
                    sq_t = sqp.tile([128, NT], BF16, tag="sq", name="sq_t")
                    nc.gpsimd.tensor_tensor(sq_t, sl, sl, op=OP.mult)
                    nc.tensor.matmul(sum_ps, ones_bf, sl,
                                     start=(k == 0), stop=(k == KH - 1))
                    nc.tensor.matmul(sq_ps, ones_bf, sq_t,
                                     start=(k == 0), stop=(k == KH - 1))
                p["st1"] = (sum_ps, sq_ps)

            return [quarter(0), quarter(4), quarter(8), quarter(12), stats]

        def ln_apply(sum_ps, sq_ps, t_s, h_s, ktiles, gvec_off, bias_off, bg_s):
            inv_n = 1.0 / (ktiles * 128)
            mean_s = stp.tile([1, NT], F32, tag="st", name="mean_s")
            nc.scalar.activation(mean_s, sum_ps, AF.Copy, scale=inv_n)
            ex2_s = stp.tile([1, NT], F32, tag="st", name="ex2_s")
            nc.scalar.activation(ex2_s, sq_ps, AF.Identity, bias=epsb, scale=inv_n)
            msq_s = stp.tile([1, NT], F32, tag="st", name="msq_s")
            nc.vector.tensor_tensor(msq_s, mean_s, mean_s, op=OP.mult)
            vpe_s = stp.tile([1, NT], F32, tag="st", name="vpe_s")
            nc.vector.tensor_tensor(vpe_s, ex2_s, msq_s, op=OP.subtract)
            sd_s = stp.tile([1, NT], F32, tag="st", name="sd_s")
            nc.scalar.activation(sd_s, vpe_s, AF.Sqrt)
            rstd_s = stp.tile([1, NT], BF16, tag="st", name="rstd_s")
            nmr_s = stp.tile([1, NT], BF16, tag="st", name="nmr_s")
            with nc.allow_low_precision(reason="LN vectors feed bf16 bcast mm"):
                nc.vector.reciprocal(rstd_s, sd_s)
                nc.vector.scalar_tensor_tensor(
                    nmr_s, mean_s, -1.0, rstd_s, op0=OP.mult, op1=OP.mult)
            bc_ps = auxps.tile([128, NT], F32, tag="aux", name="bc_ps")
            nc.tensor.matmul(bc_ps, ones_r, rstd_s, start=True, stop=True)
            rstd_b = bcp.tile([128, NT], F32, tag="bc", name="rstd_b")
            nc.scalar.activation(rstd_b, bc_ps, AF.Copy)
            bc2_ps = auxps.tile([128, NT], F32, tag="aux", name="bc2_ps")
            nc.tensor.matmul(bc2_ps, ones_r, nmr_s, start=True, stop=True)
            nmr_b = bcp.tile([128, NT], F32, tag="bc", name="nmr_b")
            nc.scalar.activation(nmr_b, bc2_ps, AF.Copy)
            for k in range(ktiles):
                x1_t = nscp.tile([128, NT], F32, tag="nsc", name="x1_t")
                nc.vector.tensor_tensor(x1_t, t_s[:, ts(k, NT)], rstd_b,
                                        op=OP.mult)
                x2_t = nscp.tile([128, NT], F32, tag="nsc", name="x2_t")
                nc.vector.tensor_tensor(x2_t, x1_t, nmr_b, op=OP.add)
                nc.scalar.activation(
                    h_s[:, ts(k, NT)], x2_t, AF.Gelu,
                    bias=bg_s[:, bias_off + k:bias_off + k + 1],
                    scale=bg_s[:, gvec_off + k:gvec_off + k + 1],
                )

        def finish_encoder(p):
            cols = p["cols"]
            zin_s = p["zin_s"]
            zbf_s = p["zbf_s"]
            t1_s = p["t1_s"]
            sum1, sq1 = p["st1"]
            h1_s = h1p.tile([128, KH * NT], BF16, tag="h1", name="h1_s")
            ln_apply(sum1, sq1, t1_s, h1_s, KH, KH, 2 * KH, b1g_s)

            t2_s = tp.tile([128, KH * NT], BF16, tag="t", name="t2_s")
            for q0 in range(0, KH, 4):
                pss = []
                for mi in range(4):
                    ps = mmps.tile([128, NT], F32, tag="mm", name="ps")
                    pss.append(ps)
                for kg in range(0, KH, 4):
                    wt = w2p.tile([128, 4, 4 * 128], BF16, tag="w2", name="wt")
                    nc.sync.dma_start(
                        out=wt,
                        in_=w2_r[:, kg:kg + 4, q0 * 128:(q0 + 4) * 128],
                    )
                    for dk in range(4):
                        k = kg + dk
                        for mi in range(4):
                            nc.tensor.matmul(
                                pss[mi], wt[:, dk, ts(mi, 128)],
                                h1_s[:, ts(k, NT)],
                                start=(k == 0), stop=(k == KH - 1),
                            )
                for mi in range(4):
                    m = q0 + mi
                    nc.scalar.activation(t2_s[:, ts(m, NT)], pss[mi], AF.Identity,
                                         bias=b2g_s[:, m:m + 1])
            sum2 = auxps.tile([1, NT], F32, tag="aux", name="sum2")
            sq2_ps = auxps.tile([1, NT], F32, tag="aux", name="sq2_ps")
            for k in range(KH):
                sl = t2_s[:, ts(k, NT)]
                sq_t = sqp.tile([128, NT], BF16, tag="sq", name="sq_t")
                nc.gpsimd.tensor_tensor(sq_t, sl, sl, op=OP.mult)
                nc.tensor.matmul(sum2, ones_bf, sl,
                                 start=(k == 0), stop=(k == KH - 1))
                nc.tensor.matmul(sq2_ps, ones_bf, sq_t,
                                 start=(k == 0), stop=(k == KH - 1))
            h2_s = h2p.tile([128, KH * NT], BF16, tag="h2", name="h2_s")
            ln_apply(sum2, sq2_ps, t2_s, h2_s, KH, KH, 2 * KH, b2g_s)

            m1_s = m1p.tile([128, KC * NT], F32, tag="m1", name="m1_s")
            val_s = valp.tile([128, KC * NT], F32, tag="val", name="val_s")
            mask_s = maskp.tile([128, KC * NT], F32, tag="mask", name="mask_s")

            def mv_drain(m, ps):
                if m < KC:
                    nc.scalar.activation(m1_s[:, ts(m, NT)], ps, AF.Sigmoid,
                                         bias=bmv_s[:, m:m + 1], scale=-1.0)
                    nc.vector.tensor_scalar(
                        mask_s[:, ts(m, NT)], m1_s[:, ts(m, NT)],
                        -1.0, 1.0, op0=OP.mult, op1=OP.add)
                else:
                    mm = m - KC
                    nc.scalar.activation(val_s[:, ts(mm, NT)], ps, AF.Identity,
                                         bias=bmv_s[:, KC + mm:KC + mm + 1])

            mm_quarters(
                2 * KC, KH,
                lambda k, m: wmv_s[:, k, ts(m, 128)],
                lambda k: h2_s[:, ts(k, NT)],
                mv_drain,
            )
            nc.sync.dma_start(
                out=maskT_r[:, :, cols],
                in_=mask_s.rearrange("p (m n) -> p m n", m=KC),
            )

            eps_s = epsp.tile([128, KC * NT], F32, tag="eps", name="eps_s")

            def eps_drain(m, ps):
                nc.vector.tensor_tensor(
                    eps_s[:, ts(m, NT)], zin_s[:, ts(m, NT)], ps, op=OP.subtract)

            mm_quarters(
                KC, KC,
                lambda k, m: adj_s[:, k, ts(m, 128)],
                lambda k: zbf_s[:, ts(k, NT)],
                eps_drain,
            )
            nc.sync.dma_start(
                out=epsT_r[:, :, cols],
                in_=eps_s.rearrange("p (m n) -> p m n", m=KC),
            )

            z0_s = zp.tile([128, KC * NT], BF16, tag="z", name="z0_s")
            c_s = cp.tile([128, KC * NT], F32, tag="c", name="c_s")
            for m in range(KC):
                sl = ts(m, NT)
                d1 = fscp.tile([128, NT], F32, tag="fsc2", name="d1")
                nc.gpsimd.tensor_tensor(d1, zin_s[:, sl], val_s[:, sl],
                                        op=OP.subtract)
                p1 = fscp.tile([128, NT], F32, tag="fsc2", name="p1")
                nc.vector.tensor_tensor(p1, d1, m1_s[:, sl], op=OP.mult)
                z0f = fscp.tile([128, NT], F32, tag="fsc2", name="z0f")
                nc.vector.tensor_tensor(z0f, val_s[:, sl], p1, op=OP.add)
                nc.gpsimd.tensor_copy(z0_s[:, sl], z0f)
                e2 = fscp.tile([128, NT], F32, tag="fsc2", name="e2")
                nc.gpsimd.tensor_tensor(e2, eps_s[:, sl], z0f, op=OP.subtract)
                p2 = fscp.tile([128, NT], F32, tag="fsc2", name="p2")
                nc.vector.tensor_tensor(p2, e2, m1_s[:, sl], op=OP.mult)
                nc.vector.tensor_tensor(c_s[:, sl], z0f, p2, op=OP.add)
            return dict(cols=cols, m1_s=m1_s, c_s=c_s, z_cur=z0_s)

        def tile_step(st, t):
            cols = st["cols"]
            m1_s = st["m1_s"]
            c_s = st["c_s"]
            z_cur = st["z_cur"]
            z_new = zp.tile([128, KC * NT], BF16, tag="z", name="z_new")
            delta_s = dltp.tile([128, KC * NT], BF16, tag="dlt", name="delta_s")

            def z_drain(m, ps):
                sl = ts(m, NT)
                p1 = fscp.tile([128, NT], F32, tag="fsc2", name="zp1")
                nc.vector.tensor_tensor(p1, ps, m1_s[:, sl], op=OP.mult)
                nc.vector.tensor_tensor(z_new[:, sl], c_s[:, sl], p1, op=OP.add)
                ds_ = fscp.tile([128, NT], BF16, tag="dsc", name="ds_")
                nc.gpsimd.tensor_tensor(ds_, z_new[:, sl], z_cur[:, sl],
                                        op=OP.subtract)
                nc.scalar.activation(delta_s[:, sl], ds_, AF.Abs)

            mm_quarters(
                KC, KC,
                lambda k, m: adj_s[:, k, ts(m, 128)],
                lambda k: z_cur[:, ts(k, NT)],
                z_drain,
            )
            nc.sync.dma_start(
                out=zallT_r[t, :, :, cols],
                in_=z_new.rearrange("p (m n) -> p m n", m=KC),
            )

            g_s = gp.tile([128, KC * NT], BF16, tag="g", name="g_s")

            def g_drain(m, ps):
                nc.scalar.activation(g_s[:, ts(m, NT)], ps, AF.Gelu,
                                     bias=cst_s[:, t * KC + m:t * KC + m + 1])

            mm_quarters(
                KC, KC,
                lambda k, m: wh1_s[:, k, ts(m, 128)],
                lambda k: delta_s[:, ts(k, NT)],
                g_drain,
            )

            y_ps = auxps.tile([1, NT], F32, tag="aux", name="y_ps")
            for k in range(KC):
                nc.tensor.matmul(
                    y_ps, wh2_s[:, k:k + 1], g_s[:, ts(k, NT)],
                    start=(k == 0), stop=(k == KC - 1),
                )
            y_row = yp.tile([1, NT], F32, tag="y", name="y_row")
            nc.scalar.activation(y_row, y_ps, AF.Copy)
            nc.sync.dma_start(out=y_o[t:t + 1, cols], in_=y_row)
            st["z_cur"] = z_new

        p = preload(0)
        chunks = l1_chunks(p)
        for ch in chunks:
            ch()
        for j in range(NTILES):
            sj = finish_encoder(p)
            if j + 1 < NTILES:
                p = preload(j + 1)
                chunks = l1_chunks(p)
            else:
                chunks = []
            for t in range(STEPS):
                tile_step(sj, t)
                if t < len(chunks):
                    chunks[t]()


_built = None


def _get_nc():
    global _built
    if _built is None:
        _built = build_nc()
    return _built


def kernel(**inputs):
    global LAST_EXEC_NS, LAST_RESULTS
    inp = {k: np.asarray(v) for k, v in inputs.items()}

    w1 = inp["w1"].astype(BF16NP)
    w2 = inp["w2"].astype(BF16NP)
    wmv = np.concatenate([inp["wm"], inp["wv"]], axis=1).astype(BF16NP)
    adj = inp["adjacency"].astype(BF16NP)
    wh1a = inp["wh1"][:DC].astype(BF16NP)
    wh2 = inp["wh2"].astype(BF16NP)
    b1g = np.concatenate(
        [_pcol(inp["b1"]), _pcol(inp["g1"]), _pcol(inp["be1"])], axis=1)
    b2g = np.concatenate(
        [_pcol(inp["b2"]), _pcol(inp["g2"]), _pcol(inp["be2"])], axis=1)
    bmv = np.concatenate([_pcol(-inp["bm"]), _pcol(inp["bv"])], axis=1)
    cst_cols = []
    wh1b = inp["wh1"][DC:].astype(np.float64)
    bh1 = inp["bh1"].astype(np.float64)
    for t in range(STEPS):
        enc = _sin_enc(float(t), ENC_D)
        cst_cols.append(_pcol((enc @ wh1b + bh1).astype(np.float32)))
    cst = np.concatenate(cst_cols, axis=1)

    shared = dict(w1=w1, w2=w2, wmv=wmv, adj=adj, wh1a=wh1a, wh2=wh2,
                  b1g=b1g, b2g=b2g, bmv=bmv, cst=cst)

    in_maps = []
    for c in range(NCORES):
        rows = slice(c * BSH, (c + 1) * BSH)
        m = dict(shared)
        m["actionT"] = np.ascontiguousarray(inp["action"][rows].T).astype(BF16NP)
        zt = np.ascontiguousarray(inp["z"][rows].T.astype(np.float32))
        m["zT"] = zt
        m["zTbf"] = zt.astype(BF16NP)
        in_maps.append(m)

    nc = _get_nc()
    res = run_bass_kernel_spmd(
        nc, in_maps, core_ids=list(range(NCORES)), trace=TRACE
    )
    LAST_EXEC_NS = res.exec_time_ns
    LAST_RESULTS = res
    outs = res.results

    # ---- host-side unshard + halt decision --------------------------------
    bh2 = float(np.asarray(inp["bh2"]).reshape(-1)[0])
    y_full = np.concatenate([np.asarray(outs[c]["yout"]) for c in range(NCORES)],
                            axis=1)  # [6, B]
    hp = 1.0 / (1.0 + np.exp(-(y_full.astype(np.float64) + bh2)))
    hmeans = hp.mean(axis=1)
    T = STEPS
    for t in range(STEPS):
        if hmeans[t] > 0.5:
            T = t + 1
            break

    mask = np.empty((B, DC), np.float32)
    epsilon = np.empty((B, DC), np.float32)
    z_state = np.empty((B, DC), np.float32)
    for c in range(NCORES):
        rows = slice(c * BSH, (c + 1) * BSH)
        mask[rows] = np.asarray(outs[c]["maskT"]).T
        epsilon[rows] = np.asarray(outs[c]["epsT"]).T
        z_state[rows] = np.asarray(outs[c]["zallT"])[T - 1].T.astype(np.float32)

    return z_state, mask, epsilon, np.int32(T)


# revision 38
# speedup vs baseline: 1.3335x; 1.1025x over previous
"""Trainium2 Bass kernel for nn_AdaptiveInterventionEngine.

Data-parallel over batch across 8 NeuronCores. Feature-major (transposed)
activation layout on device so every weight matrix is used as lhsT exactly
as stored. Encoder matmuls bf16 (fp32 PSUM accumulate), LayerNorm stats in
fp32 via ones-vector matmuls on the PE, recurrence state in bf16 with fp32
elementwise updates. The per-step halt decision is decoupled: the device computes the
full 6-step trajectory plus per-element halter logits; the host reduces the
logit means, derives steps_used and selects the matching z state.
"""

import math

import numpy as np
import ml_dtypes

import concourse.bass as bass
import concourse.mybir as mybir
import concourse.tile as tile
from concourse import bacc
from concourse.bass import ts
from concourse.bass_utils import run_bass_kernel_spmd

BF16NP = ml_dtypes.bfloat16
F32 = mybir.dt.float32
F32R = mybir.dt.float32r
BF16 = mybir.dt.bfloat16
AF = mybir.ActivationFunctionType
OP = mybir.AluOpType

NCORES = 8
B = 32768
BSH = B // NCORES          # 4096 batch rows per core
DC = 512                   # causal dim
DA = 1024                  # action dim
HID = 2048                 # hidden dim
ENC_D = DC // 4            # 128
STEPS = 6
NT = 256                   # batch columns per tile
NTILES = BSH // NT         # 16
KA = DA // 128             # 8  action k-chunks
KH = HID // 128            # 16 hidden k-chunks
KC = DC // 128             # 4  causal k-chunks

TRACE = False
LAST_EXEC_NS = None
LAST_RESULTS = None


def _sin_enc(step_f, d):
    div = np.exp(np.arange(0, d, 2, dtype=np.float64) * (-(math.log(10000.0) / d)))
    pe = np.zeros((d,), dtype=np.float64)
    pe[0::2] = np.sin(step_f * div[: (d + 1) // 2])
    pe[1::2] = np.cos(step_f * div[: d // 2])
    return pe


def _pcol(v):
    """[n*128] vector -> [128, n] (partition, m-chunk) layout, f32."""
    v = np.asarray(v, np.float32)
    n = v.shape[0] // 128
    return np.ascontiguousarray(v.reshape(n, 128).T)


def build_nc():
    nc = bacc.Bacc(
        "TRN2", target_bir_lowering=False, debug=False, num_devices=NCORES
    )

    def din(name, shape, dt):
        return nc.dram_tensor(name, shape, dt, kind="ExternalInput").ap()

    def dout(name, shape, dt):
        return nc.dram_tensor(name, shape, dt, kind="ExternalOutput").ap()

    actT_d = din("actionT", [DA, BSH], BF16)
    zT_d = din("zT", [DC, BSH], F32)
    zbf_d = din("zTbf", [DC, BSH], BF16)
    w1_d = din("w1", [DA, HID], BF16)
    w2_d = din("w2", [HID, HID], BF16)
    wmv_d = din("wmv", [HID, 2 * DC], BF16)
    adj_d = din("adj", [DC, DC], BF16)
    wh1_d = din("wh1a", [DC, DC], BF16)
    wh2_d = din("wh2", [DC, 1], BF16)
    b1g_d = din("b1g", [128, 3 * KH], F32)    # b1 | g1 | be1
    b2g_d = din("b2g", [128, 3 * KH], F32)    # b2 | g2 | be2
    bmv_d = din("bmv", [128, 2 * KC], F32)    # -bm | bv
    cst_d = din("cst", [128, STEPS * KC], F32)  # halter per-step bias

    maskT_o = dout("maskT", [DC, BSH], F32)
    epsT_o = dout("epsT", [DC, BSH], F32)
    zallT_o = dout("zallT", [STEPS, DC, BSH], BF16)
    y_o = dout("yout", [STEPS, BSH], F32)

    with tile.TileContext(nc) as tc:
        build_kernel(
            nc, tc,
            actT_d, zT_d, zbf_d, w1_d, w2_d, wmv_d, adj_d, wh1_d, wh2_d,
            b1g_d, b2g_d, bmv_d, cst_d,
            maskT_o, epsT_o, zallT_o, y_o,
        )
    nc.compile()
    return nc


def build_kernel(nc, tc, actT_d, zT_d, zbf_d, w1_d, w2_d, wmv_d, adj_d, wh1_d, wh2_d,
                 b1g_d, b2g_d, bmv_d, cst_d, maskT_o, epsT_o, zallT_o, y_o):
    from contextlib import ExitStack

    ctx = ExitStack()
    with ctx:
        singles = ctx.enter_context(tc.tile_pool(name="singles", bufs=1))
        w2p = ctx.enter_context(tc.tile_pool(name="w2p", bufs=3))
        actp = ctx.enter_context(tc.tile_pool(name="actp", bufs=2))
        tp = ctx.enter_context(tc.tile_pool(name="tp", bufs=2))
        h1p = ctx.enter_context(tc.tile_pool(name="h1p", bufs=2))
        h2p = ctx.enter_context(tc.tile_pool(name="h2p", bufs=2))
        sqp = ctx.enter_context(tc.tile_pool(name="sqp", bufs=2))
        stp = ctx.enter_context(tc.tile_pool(name="stp", bufs=6))
        bcp = ctx.enter_context(tc.tile_pool(name="bcp", bufs=2))
        nscp = ctx.enter_context(tc.tile_pool(name="nscp", bufs=3))
        zinp = ctx.enter_context(tc.tile_pool(name="zinp", bufs=2))
        zbfp = ctx.enter_context(tc.tile_pool(name="zbfp", bufs=1))
        m1p = ctx.enter_context(tc.tile_pool(name="m1p", bufs=2))
        valp = ctx.enter_context(tc.tile_pool(name="valp", bufs=1))
        epsp = ctx.enter_context(tc.tile_pool(name="epsp", bufs=1))
        maskp = ctx.enter_context(tc.tile_pool(name="maskp", bufs=1))
        fscp = ctx.enter_context(tc.tile_pool(name="fscp", bufs=4))
        cp = ctx.enter_context(tc.tile_pool(name="cp", bufs=2))
        zp = ctx.enter_context(tc.tile_pool(name="zp", bufs=3))
        dltp = ctx.enter_context(tc.tile_pool(name="dltp", bufs=1))
        gp = ctx.enter_context(tc.tile_pool(name="gp", bufs=1))
        yp = ctx.enter_context(tc.tile_pool(name="yp", bufs=1))
        mmps = ctx.enter_context(tc.tile_pool(name="mmps", bufs=5, space="PSUM"))
        auxps = ctx.enter_context(tc.tile_pool(name="auxps", bufs=3, space="PSUM"))

        # ---- resident weights -------------------------------------------
        w1_s = singles.tile([128, KA, HID], BF16, name="w1_s")
        nc.sync.dma_start(out=w1_s, in_=w1_d.rearrange("(k p) m -> p k m", p=128))
        wmv_s = singles.tile([128, KH, 2 * DC], BF16, name="wmv_s")
        nc.sync.dma_start(out=wmv_s, in_=wmv_d.rearrange("(k p) m -> p k m", p=128))
        adj_s = singles.tile([128, KC, DC], BF16, name="adj_s")
        nc.sync.dma_start(out=adj_s, in_=adj_d.rearrange("(k p) m -> p k m", p=128))
        wh1_s = singles.tile([128, KC, DC], BF16, name="wh1_s")
        nc.sync.dma_start(out=wh1_s, in_=wh1_d.rearrange("(k p) m -> p k m", p=128))
        wh2_s = singles.tile([128, KC], BF16, name="wh2_s")
        nc.sync.dma_start(out=wh2_s, in_=wh2_d.rearrange("(k p) o -> p (k o)", p=128))
        b1g_s = singles.tile([128, 3 * KH], F32, name="b1g_s")
        nc.sync.dma_start(out=b1g_s, in_=b1g_d)
        b2g_s = singles.tile([128, 3 * KH], F32, name="b2g_s")
        nc.sync.dma_start(out=b2g_s, in_=b2g_d)
        bmv_s = singles.tile([128, 2 * KC], F32, name="bmv_s")
        nc.sync.dma_start(out=bmv_s, in_=bmv_d)
        cst_s = singles.tile([128, STEPS * KC], F32, name="cst_s")
        nc.sync.dma_start(out=cst_s, in_=cst_d)
        ones_bf = singles.tile([128, 1], BF16, name="ones_bf")
        nc.vector.memset(ones_bf, 1.0)
        ones_r = singles.tile([1, 128], BF16, name="ones_r")
        nc.vector.memset(ones_r, 1.0)
        epsb = singles.tile([1, 1], F32, name="epsb")
        nc.vector.memset(epsb, 1e-5)

        actT_r = actT_d.rearrange("(k p) n -> p k n", p=128)
        zT_r = zT_d.rearrange("(k p) n -> p k n", p=128)
        zbf_r = zbf_d.rearrange("(k p) n -> p k n", p=128)
        maskT_r = maskT_o.rearrange("(m p) n -> p m n", p=128)
        epsT_r = epsT_o.rearrange("(m p) n -> p m n", p=128)
        zallT_r = zallT_o.rearrange("t (m p) n -> t p m n", p=128)

        def bcast2(ap2d, n):
            return bass.AP(tensor=ap2d.tensor, offset=ap2d.offset,
                           ap=[ap2d.ap[0], [0, n], ap2d.ap[1]])

        def mm_quarters(M, K, lhsT_fn, rhs_fn, drain_fn, q=4):
            """out_mtile[m] = sum_k lhsT(k,m).T @ rhs(k); M,K in 128-tiles."""
            for q0 in range(0, M, q):
                nq = min(q, M - q0)
                pss = []
                for mi in range(nq):
                    ps = mmps.tile([128, NT], F32, tag="mm", name="ps")
                    pss.append(ps)
                for k in range(K):
                    for mi in range(nq):
                        nc.tensor.matmul(
                            pss[mi], lhsT_fn(k, q0 + mi), rhs_fn(k),
                            start=(k == 0), stop=(k == K - 1),
                        )
                for mi in range(nq):
                    drain_fn(q0 + mi, pss[mi])

        w2_r = w2_d.rearrange("(k p) m -> p k m", p=128)

        def preload(j):
            cols = slice(j * NT, (j + 1) * NT)
            a_s = actp.tile([128, KA * NT], BF16, tag="act", name="a_s")
            nc.sync.dma_start(
                out=a_s.rearrange("p (k n) -> p k n", k=KA),
                in_=actT_r[:, :, cols],
            )
            zin_s = zinp.tile([128, KC * NT], F32, tag="zin", name="zin_s")
            nc.sync.dma_start(
                out=zin_s.rearrange("p (k n) -> p k n", k=KC),
                in_=zT_r[:, :, cols],
            )
            zbf_s = zbfp.tile([128, KC * NT], BF16, tag="zbf", name="zbf_s")
            nc.sync.dma_start(
                out=zbf_s.rearrange("p (k n) -> p k n", k=KC),
                in_=zbf_r[:, :, cols],
            )
            return dict(cols=cols, a_s=a_s, zin_s=zin_s, zbf_s=zbf_s)

        def l1_chunks(p):
            """Closures emitting L1 quarters + stats for one tile."""
            a_s = p["a_s"]
            t1_s = tp.tile([128, KH * NT], BF16, tag="t", name="t1_s")
            p["t1_s"] = t1_s

            def l1_drain(m, ps):
                nc.scalar.activation(t1_s[:, ts(m, NT)], ps, AF.Identity,
                                     bias=b1g_s[:, m:m + 1])

            def quarter(q0):
                def run():
                    pss = []
                    for mi in range(4):
                        psq = mmps.tile([128, NT], F32, tag="mm", name="ps")
                        pss.append(psq)
                    for k in range(KA):
                        for mi in range(4):
                            nc.tensor.matmul(
                                pss[mi], w1_s[:, k, ts(q0 + mi, 128)],
                                a_s[:, ts(k, NT)],
                                start=(k == 0), stop=(k == KA - 1),
                            )
                    for mi in range(4):
                        l1_drain(q0 + mi, pss[mi])
                return run

            def stats_half(k0, k1):
                def run():
                    if k0 == 0:
                        sum_ps = auxps.tile([1, NT], F32, tag="aux",
                                            name="sum_ps")
                        sq_ps = auxps.tile([1, NT], F32, tag="aux",
                                           name="sq_ps")
                        p["st1"] = (sum_ps, sq_ps)
                    sum_ps, sq_ps = p["st1"]
                    for k in range(k0, k1):
                        sl = t1_s[:, ts(k, NT)]
                        sq_t = sqp.tile([128, NT], BF16, tag="sq", name="sq_t")
                        nc.gpsimd.tensor_tensor(sq_t, sl, sl, op=OP.mult)
                        nc.tensor.matmul(sum_ps, ones_bf, sl,
                                         start=(k == 0), stop=(k == KH - 1))
                        nc.tensor.matmul(sq_ps, ones_bf, sq_t,
                                         start=(k == 0), stop=(k == KH - 1))
                return run

            def ln1():
                sum_ps, sq_ps = p["st1"]
                h1_s = h1p.tile([128, KH * NT], BF16, tag="h1", name="h1_s")
                ln_apply(sum_ps, sq_ps, t1_s, h1_s, KH, KH, 2 * KH, b1g_s)
                p["h1_s"] = h1_s

            return [quarter(0), quarter(4), quarter(8), quarter(12),
                    stats_half(0, KH // 2), stats_half(KH // 2, KH), ln1]

        def ln_apply(sum_ps, sq_ps, t_s, h_s, ktiles, gvec_off, bias_off, bg_s):
            inv_n = 1.0 / (ktiles * 128)
            mean_s = stp.tile([1, NT], F32, tag="st", name="mean_s")
            nc.scalar.activation(mean_s, sum_ps, AF.Copy, scale=inv_n)
            ex2_s = stp.tile([1, NT], F32, tag="st", name="ex2_s")
            nc.scalar.activation(ex2_s, sq_ps, AF.Identity, bias=epsb, scale=inv_n)
            msq_s = stp.tile([1, NT], F32, tag="st", name="msq_s")
            nc.vector.tensor_tensor(msq_s, mean_s, mean_s, op=OP.mult)
            vpe_s = stp.tile([1, NT], F32, tag="st", name="vpe_s")
            nc.vector.tensor_tensor(vpe_s, ex2_s, msq_s, op=OP.subtract)
            lnv_s = stp.tile([1, NT], F32, tag="st", name="lnv_s")
            nc.scalar.activation(lnv_s, vpe_s, AF.Ln)
            rstd_s = stp.tile([1, NT], BF16, tag="st", name="rstd_s")
            nc.scalar.activation(rstd_s, lnv_s, AF.Exp, scale=-0.5)
            nmr_s = stp.tile([1, NT], BF16, tag="st", name="nmr_s")
            with nc.allow_low_precision(reason="LN vectors feed bf16 bcast mm"):
                nc.vector.scalar_tensor_tensor(
                    nmr_s, mean_s, -1.0, rstd_s, op0=OP.mult, op1=OP.mult)
            bc_ps = auxps.tile([128, NT], F32, tag="aux", name="bc_ps")
            nc.tensor.matmul(bc_ps, ones_r, rstd_s, start=True, stop=True)
            rstd_b = bcp.tile([128, NT], F32, tag="bc", name="rstd_b")
            nc.scalar.activation(rstd_b, bc_ps, AF.Copy)
            bc2_ps = auxps.tile([128, NT], F32, tag="aux", name="bc2_ps")
            nc.tensor.matmul(bc2_ps, ones_r, nmr_s, start=True, stop=True)
            nmr_b = bcp.tile([128, NT], F32, tag="bc", name="nmr_b")
            nc.scalar.activation(nmr_b, bc2_ps, AF.Copy)
            for k in range(ktiles):
                x1_t = nscp.tile([128, NT], F32, tag="nsc", name="x1_t")
                nc.vector.tensor_tensor(x1_t, t_s[:, ts(k, NT)], rstd_b,
                                        op=OP.mult)
                x2_t = nscp.tile([128, NT], F32, tag="nsc", name="x2_t")
                nc.vector.tensor_tensor(x2_t, x1_t, nmr_b, op=OP.add)
                nc.scalar.activation(
                    h_s[:, ts(k, NT)], x2_t, AF.Gelu,
                    bias=bg_s[:, bias_off + k:bias_off + k + 1],
                    scale=bg_s[:, gvec_off + k:gvec_off + k + 1],
                )

        def finish_encoder(p):
            cols = p["cols"]
            zin_s = p["zin_s"]
            zbf_s = p["zbf_s"]
            h1_s = p["h1_s"]

            t2_s = tp.tile([128, KH * NT], BF16, tag="t", name="t2_s")
            for q0 in range(0, KH, 4):
                pss = []
                for mi in range(4):
                    ps = mmps.tile([128, NT], F32, tag="mm", name="ps")
                    pss.append(ps)
                for kg in range(0, KH, 4):
                    wt = w2p.tile([128, 4, 4 * 128], BF16, tag="w2", name="wt")
                    nc.sync.dma_start(
                        out=wt,
                        in_=w2_r[:, kg:kg + 4, q0 * 128:(q0 + 4) * 128],
                    )
                    for dk in range(4):
                        k = kg + dk
                        for mi in range(4):
                            nc.tensor.matmul(
                                pss[mi], wt[:, dk, ts(mi, 128)],
                                h1_s[:, ts(k, NT)],
                                start=(k == 0), stop=(k == KH - 1),
                            )
                for mi in range(4):
                    m = q0 + mi
                    nc.scalar.activation(t2_s[:, ts(m, NT)], pss[mi], AF.Identity,
                                         bias=b2g_s[:, m:m + 1])
            sum2 = auxps.tile([1, NT], F32, tag="aux", name="sum2")
            sq2_ps = auxps.tile([1, NT], F32, tag="aux", name="sq2_ps")
            for k in range(KH):
                sl = t2_s[:, ts(k, NT)]
                sq_t = sqp.tile([128, NT], BF16, tag="sq", name="sq_t")
                nc.gpsimd.tensor_tensor(sq_t, sl, sl, op=OP.mult)
                nc.tensor.matmul(sum2, ones_bf, sl,
                                 start=(k == 0), stop=(k == KH - 1))
                nc.tensor.matmul(sq2_ps, ones_bf, sq_t,
                                 start=(k == 0), stop=(k == KH - 1))

            # epsilon matmuls are independent — emit before the LN2 chain so
            # the PE has work while the chain resolves on DVE/ACT
            eps_s = epsp.tile([128, KC * NT], F32, tag="eps", name="eps_s")

            def eps_drain(m, ps):
                nc.vector.tensor_tensor(
                    eps_s[:, ts(m, NT)], zin_s[:, ts(m, NT)], ps, op=OP.subtract)

            mm_quarters(
                KC, KC,
                lambda k, m: adj_s[:, k, ts(m, 128)],
                lambda k: zbf_s[:, ts(k, NT)],
                eps_drain,
            )
            nc.sync.dma_start(
                out=epsT_r[:, :, cols],
                in_=eps_s.rearrange("p (m n) -> p m n", m=KC),
            )

            h2_s = h2p.tile([128, KH * NT], BF16, tag="h2", name="h2_s")
            ln_apply(sum2, sq2_ps, t2_s, h2_s, KH, KH, 2 * KH, b2g_s)

            m1_s = m1p.tile([128, KC * NT], F32, tag="m1", name="m1_s")
            val_s = valp.tile([128, KC * NT], F32, tag="val", name="val_s")
            mask_s = maskp.tile([128, KC * NT], F32, tag="mask", name="mask_s")

            def mv_drain(m, ps):
                if m < KC:
                    nc.scalar.activation(m1_s[:, ts(m, NT)], ps, AF.Sigmoid,
                                         bias=bmv_s[:, m:m + 1], scale=-1.0)
                    nc.vector.tensor_scalar(
                        mask_s[:, ts(m, NT)], m1_s[:, ts(m, NT)],
                        -1.0, 1.0, op0=OP.mult, op1=OP.add)
                else:
                    mm = m - KC
                    nc.scalar.activation(val_s[:, ts(mm, NT)], ps, AF.Identity,
                                         bias=bmv_s[:, KC + mm:KC + mm + 1])

            mm_quarters(
                2 * KC, KH,
                lambda k, m: wmv_s[:, k, ts(m, 128)],
                lambda k: h2_s[:, ts(k, NT)],
                mv_drain,
            )
            nc.sync.dma_start(
                out=maskT_r[:, :, cols],
                in_=mask_s.rearrange("p (m n) -> p m n", m=KC),
            )

            z0_s = zp.tile([128, KC * NT], BF16, tag="z", name="z0_s")
            c_s = cp.tile([128, KC * NT], F32, tag="c", name="c_s")
            for m in range(KC):
                sl = ts(m, NT)
                d1 = fscp.tile([128, NT], F32, tag="fsc2", name="d1")
                nc.gpsimd.tensor_tensor(d1, zin_s[:, sl], val_s[:, sl],
                                        op=OP.subtract)
                p1 = fscp.tile([128, NT], F32, tag="fsc2", name="p1")
                nc.vector.tensor_tensor(p1, d1, m1_s[:, sl], op=OP.mult)
                z0f = fscp.tile([128, NT], F32, tag="fsc2", name="z0f")
                nc.vector.tensor_tensor(z0f, val_s[:, sl], p1, op=OP.add)
                nc.gpsimd.tensor_copy(z0_s[:, sl], z0f)
                e2 = fscp.tile([128, NT], F32, tag="fsc2", name="e2")
                nc.gpsimd.tensor_tensor(e2, eps_s[:, sl], z0f, op=OP.subtract)
                p2 = fscp.tile([128, NT], F32, tag="fsc2", name="p2")
                nc.vector.tensor_tensor(p2, e2, m1_s[:, sl], op=OP.mult)
                nc.vector.tensor_tensor(c_s[:, sl], z0f, p2, op=OP.add)
            return dict(cols=cols, m1_s=m1_s, c_s=c_s, z_cur=z0_s)

        def step_z(st, t):
            cols = st["cols"]
            m1_s = st["m1_s"]
            c_s = st["c_s"]
            z_cur = st["z_cur"]
            z_new = zp.tile([128, KC * NT], BF16, tag="z", name="z_new")
            delta_s = dltp.tile([128, KC * NT], BF16, tag="dlt", name="delta_s")

            def z_drain(m, ps):
                sl = ts(m, NT)
                p1 = fscp.tile([128, NT], F32, tag="fsc2", name="zp1")
                nc.vector.tensor_tensor(p1, ps, m1_s[:, sl], op=OP.mult)
                nc.vector.tensor_tensor(z_new[:, sl], c_s[:, sl], p1, op=OP.add)
                ds_ = fscp.tile([128, NT], BF16, tag="dsc", name="ds_")
                nc.gpsimd.tensor_tensor(ds_, z_new[:, sl], z_cur[:, sl],
                                        op=OP.subtract)
                nc.scalar.activation(delta_s[:, sl], ds_, AF.Abs)

            mm_quarters(
                KC, KC,
                lambda k, m: adj_s[:, k, ts(m, 128)],
                lambda k: z_cur[:, ts(k, NT)],
                z_drain,
            )
            nc.sync.dma_start(
                out=zallT_r[t, :, :, cols],
                in_=z_new.rearrange("p (m n) -> p m n", m=KC),
            )
            st["delta_s"] = delta_s
            st["z_cur"] = z_new

        def step_h(st, t):
            cols = st["cols"]
            delta_s = st["delta_s"]
            g_s = gp.tile([128, KC * NT], BF16, tag="g", name="g_s")

            def g_drain(m, ps):
                nc.scalar.activation(g_s[:, ts(m, NT)], ps, AF.Gelu,
                                     bias=cst_s[:, t * KC + m:t * KC + m + 1])

            mm_quarters(
                KC, KC,
                lambda k, m: wh1_s[:, k, ts(m, 128)],
                lambda k: delta_s[:, ts(k, NT)],
                g_drain,
            )

            y_ps = auxps.tile([1, NT], F32, tag="aux", name="y_ps")
            for k in range(KC):
                nc.tensor.matmul(
                    y_ps, wh2_s[:, k:k + 1], g_s[:, ts(k, NT)],
                    start=(k == 0), stop=(k == KC - 1),
                )
            y_row = yp.tile([1, NT], F32, tag="y", name="y_row")
            nc.scalar.activation(y_row, y_ps, AF.Copy)
            nc.sync.dma_start(out=y_o[t:t + 1, cols], in_=y_row)

        p = preload(0)
        chunks = l1_chunks(p)
        for ch in chunks:
            ch()
        for j in range(NTILES):
            sj = finish_encoder(p)
            if j + 1 < NTILES:
                p = preload(j + 1)
                chunks = l1_chunks(p)
            else:
                chunks = []
            if chunks:
                chunks[0]()
            for t in range(STEPS):
                step_z(sj, t)
                if t + 1 < len(chunks):
                    chunks[t + 1]()
                step_h(sj, t)


_built = None


def _get_nc():
    global _built
    if _built is None:
        _built = build_nc()
    return _built


def kernel(**inputs):
    global LAST_EXEC_NS, LAST_RESULTS
    inp = {k: np.asarray(v) for k, v in inputs.items()}

    w1 = inp["w1"].astype(BF16NP)
    w2 = inp["w2"].astype(BF16NP)
    wmv = np.concatenate([inp["wm"], inp["wv"]], axis=1).astype(BF16NP)
    adj = inp["adjacency"].astype(BF16NP)
    wh1a = inp["wh1"][:DC].astype(BF16NP)
    wh2 = inp["wh2"].astype(BF16NP)
    b1g = np.concatenate(
        [_pcol(inp["b1"]), _pcol(inp["g1"]), _pcol(inp["be1"])], axis=1)
    b2g = np.concatenate(
        [_pcol(inp["b2"]), _pcol(inp["g2"]), _pcol(inp["be2"])], axis=1)
    bmv = np.concatenate([_pcol(-inp["bm"]), _pcol(inp["bv"])], axis=1)
    cst_cols = []
    wh1b = inp["wh1"][DC:].astype(np.float64)
    bh1 = inp["bh1"].astype(np.float64)
    for t in range(STEPS):
        enc = _sin_enc(float(t), ENC_D)
        cst_cols.append(_pcol((enc @ wh1b + bh1).astype(np.float32)))
    cst = np.concatenate(cst_cols, axis=1)

    shared = dict(w1=w1, w2=w2, wmv=wmv, adj=adj, wh1a=wh1a, wh2=wh2,
                  b1g=b1g, b2g=b2g, bmv=bmv, cst=cst)

    in_maps = []
    for c in range(NCORES):
        rows = slice(c * BSH, (c + 1) * BSH)
        m = dict(shared)
        m["actionT"] = np.ascontiguousarray(inp["action"][rows].T).astype(BF16NP)
        zt = np.ascontiguousarray(inp["z"][rows].T.astype(np.float32))
        m["zT"] = zt
        m["zTbf"] = zt.astype(BF16NP)
        in_maps.append(m)

    nc = _get_nc()
    res = run_bass_kernel_spmd(
        nc, in_maps, core_ids=list(range(NCORES)), trace=TRACE
    )
    LAST_EXEC_NS = res.exec_time_ns
    LAST_RESULTS = res
    outs = res.results

    # ---- host-side unshard + halt decision --------------------------------
    bh2 = float(np.asarray(inp["bh2"]).reshape(-1)[0])
    y_full = np.concatenate([np.asarray(outs[c]["yout"]) for c in range(NCORES)],
                            axis=1)  # [6, B]
    hp = 1.0 / (1.0 + np.exp(-(y_full.astype(np.float64) + bh2)))
    hmeans = hp.mean(axis=1)
    T = STEPS
    for t in range(STEPS):
        if hmeans[t] > 0.5:
            T = t + 1
            break

    mask = np.empty((B, DC), np.float32)
    epsilon = np.empty((B, DC), np.float32)
    z_state = np.empty((B, DC), np.float32)
    for c in range(NCORES):
        rows = slice(c * BSH, (c + 1) * BSH)
        mask[rows] = np.asarray(outs[c]["maskT"]).T
        epsilon[rows] = np.asarray(outs[c]["epsT"]).T
        z_state[rows] = np.asarray(outs[c]["zallT"])[T - 1].T.astype(np.float32)

    return z_state, mask, epsilon, np.int32(T)
